# revision 100
# baseline (speedup 1.0000x reference)
"""MLA (multi-head latent attention) forward on 8 TRN2 NeuronCores.

Strategy: tensor-parallel over heads (16 heads -> 2 per core), SPMD with
host-side combine of the 8 partial outputs (wo is column-sharded so partials
sum to the full output).

Kernel structure (per core):
  P1  streaming pass over x^T: latent-kv projection in bf16 (+layernorm
      with both reduction sums fused onto the Act accumulator, +rope on
      the shared key pe); q projection in fp8e4 with DoubleRow matmuls
      (4x bf16 throughput per the 0.5 cyc/row + 256-deep contraction; the
      x512 fp8 range scale is undone at PSUM eviction).  The q nope part
      is emitted directly transposed (tokens as the moving free dim); the
      remaining attention operands are transposed on the PE in bf16,
      packed into a single 1-bank PSUM tile per token chunk.
  P3  per head: materialize per-head K^T = wkc^T kv_c^T and V = kv_c wvc^T
      from the latent (cuts the score/context contraction from 576 to 192/
      128 vs the absorbed form), then block-causal attention in "scores
      transposed" [k,q] orientation with causal triangle-trim of the
      diagonal key chunks (processed largest-block-first so the leading
      pairs need no mask).  Softmax denominators come from [128,1]-output
      matmuls (~1 PE row each), reciprocals are transposed back to a row
      and broadcast across partitions on the idle GPSIMD engine, and the
      normalize is fused into the context eviction.
  P4  output projection with the per-core wo column slice in f16 output
      (host combines in fp64); batch 0's P4 is deferred until after batch
      1's P1 so its output DMAs overlap compute.

Rope pairs are host-permuted to [evens; odds] so all rope arithmetic runs
on contiguous column slices (consistently applied to q_pe and k_pe, which
leaves scores invariant).
"""

import numpy as np
from contextlib import ExitStack

import concourse.bass as bass
import concourse.tile as tile
from concourse import bacc, mybir

# ---------------- problem dims (hardcoded per contest contract) -------------
B, S, D = 2, 2048, 2048
H = 16
C = 512            # kv_lora_rank
DN, DR, DV = 128, 64, 128
NH = DN + DR       # 192
SCALE = float((DN + DR) ** -0.5)
NEG = -1e9

N_CORES = 8
HL = H // N_CORES  # 2 local heads
P = 128
DC = D // P        # 16 contraction chunks over D
TCH = S // P       # 16 token chunks per batch
NBLK = 4           # query blocks per batch
BLKQ = S // NBLK   # 512
CC = C // P        # 4 chunks over latent dim
XG = 4             # token chunks per x-block DMA
QS = 512.0         # fp8 range scale for the q projection

F32 = mybir.dt.float32
F16 = mybir.dt.float16
BF16 = mybir.dt.bfloat16
FP8 = mybir.dt.float8e4
AF = mybir.ActivationFunctionType
DROW = mybir.MatmulPerfMode.DoubleRow


def _mm(nc, out, lhsT, rhs, start, stop):
    nc.tensor.matmul(out, lhsT, rhs, start=start, stop=stop,
                     skip_group_check=True)


def _mm8(nc, out, lhsT, rhs, start, stop):
    nc.tensor.matmul(out, lhsT, rhs, start=start, stop=stop,
                     perf_mode=DROW, skip_group_check=True)


def build_nc():
    from concourse.masks import make_identity

    nc = bacc.Bacc("TRN2", target_bir_lowering=False, debug=False,
                   num_devices=N_CORES)
    x8t = nc.dram_tensor("x8t", [D, B * S], FP8, kind="ExternalInput")
    xr8t = nc.dram_tensor("xr8t", [D, B * S], FP8, kind="ExternalInput")
    wqpe8 = nc.dram_tensor("wqpe8", [D, HL * DR], FP8, kind="ExternalInput")
    wqn8 = nc.dram_tensor("wqn8", [D, HL * DN], FP8, kind="ExternalInput")
    # 3-pass split-precision kv weights: x*w ~ x8*wA + xr8*wB + x8*wC, all
    # products at a shared x512 output scale (undone at PSUM eviction);
    # reconstruction error ~1e-3, better than a bf16 matmul
    wkvaA = nc.dram_tensor("wkvaA", [D, C + DR], FP8, kind="ExternalInput")
    wkvaB = nc.dram_tensor("wkvaB", [D, C + DR], FP8, kind="ExternalInput")
    wkvaC = nc.dram_tensor("wkvaC", [D, C + DR], FP8, kind="ExternalInput")
    wkcT = nc.dram_tensor("wkcT", [HL, C, DN], BF16, kind="ExternalInput")
    wvct = nc.dram_tensor("wvct", [HL, C, DV], BF16, kind="ExternalInput")
    wot = nc.dram_tensor("wot", [HL * DV, D], BF16, kind="ExternalInput")
    cs = nc.dram_tensor("cs", [S, DR], F32, kind="ExternalInput")
    maskt = nc.dram_tensor("maskt", [P, P], BF16, kind="ExternalInput")
    # f16 output halves the out-DMA volume; the host combines in fp64 and
    # the ~0.05% rounding on per-core partials is well inside the error budget
    out = nc.dram_tensor("out", [B * S, D], F16, kind="ExternalOutput")

    with tile.TileContext(nc) as tc, ExitStack() as ctx:
        const = ctx.enter_context(tc.tile_pool(name="const", bufs=1))
        octpool = ctx.enter_context(tc.tile_pool(name="octpool", bufs=1))

        # weight / table DMAs, ordered so the P1-critical ones land first:
        #   Pool queue: wkva (split in halves) -> x-stream -> K/V weights
        #   Act queue:  wq8 -> wot -> tables
        #   SP queue:   x-stream + out
        hd = DC // 2
        qd4 = DC // 4
        wq_sb = octpool.tile([P, DC, HL * DR], FP8, name="wq_sb", tag="wq_sb")
        wqn_sb = octpool.tile([P, DC, HL, DN], FP8, name="wqn_sb", tag="wqn_sb")
        for hl in range(HL):
            nc.sync.dma_start(
                out=wqn_sb[:, :, hl, :],
                in_=wqn8[:, hl * DN:(hl + 1) * DN]
                    .rearrange("(a p) n -> p a n", p=P))
        nc.scalar.dma_start(out=wq_sb,
                            in_=wqpe8[:, :].rearrange("(a p) n -> p a n", p=P))
        # dense per-part weight tiles: DoubleRow moving APs must be dense
        # (column-sliced rhs lowers incorrectly on hardware).  The three
        # passes load from three different queues so the first x-stream
        # blocks aren't starved behind weight DMAs.
        wkvc_sb = {}
        wkpe_sb = {}
        for nm, src, eng in (("A", wkvaA, nc.gpsimd), ("B", wkvaB, nc.sync),
                             ("C", wkvaC, nc.scalar)):
            wkvc_sb[nm] = octpool.tile([P, DC, C], FP8, name=f"wkvc{nm}_sb",
                                       tag=f"wkvc{nm}_sb")
            wkpe_sb[nm] = octpool.tile([P, DC, DR], FP8, name=f"wkpe{nm}_sb",
                                       tag=f"wkpe{nm}_sb")
            if nm == "A":
                continue  # A loads after the first x8 block (see P1 loop)
            eng.dma_start(
                out=wkvc_sb[nm],
                in_=src[:, 0:C].rearrange("(a p) n -> p a n", p=P))
            eng.dma_start(
                out=wkpe_sb[nm],
                in_=src[:, C:C + DR].rearrange("(a p) n -> p a n", p=P))

        def emit_wkva_a():
            for qi in range(4):
                nc.gpsimd.dma_start(
                    out=wkvc_sb["A"][:, qi * qd4:(qi + 1) * qd4],
                    in_=wkvaA[qi * qd4 * P:(qi + 1) * qd4 * P, 0:C]
                        .rearrange("(a p) n -> p a n", p=P))
            nc.gpsimd.dma_start(
                out=wkpe_sb["A"],
                in_=wkvaA[:, C:C + DR].rearrange("(a p) n -> p a n", p=P))
        wot_sb = [octpool.tile([P, D], BF16, name=f"wot_sb{hl}", tag=f"wot{hl}")
                  for hl in range(HL)]
        outcT = [octpool.tile([P, B * S], BF16, name=f"outcT{hl}",
                              tag=f"outcT{hl}")
                 for hl in range(HL)]

        identb = const.tile([P, P], BF16, name="identb", tag="identb")
        make_identity(nc, identb)
        identh = const.tile([P, P], F16, name="identh", tag="identh")
        make_identity(nc, identh)
        ones_col = const.tile([P, 1], BF16, name="ones_col", tag="ones_col")
        nc.vector.memset(ones_col, 1.0)
        cs_sb = const.tile([P, TCH, DR], F32, name="cs_sb", tag="cs_sb")
        nc.scalar.dma_start(out=cs_sb,
                            in_=cs[:, :].rearrange("(a p) r -> p a r", p=P))
        # tables not needed until P3/P4: the DMAs are issued after the first
        # x-stream loads (emit_late_weights) so they don't delay startup
        mask_sb = const.tile([P, P], BF16, name="mask_sb", tag="mask_sb")
        wkcT_sb = const.tile([P, HL, CC, DN], BF16, name="wkcT_sb",
                             tag="wkcT_sb")
        wvct_sb = const.tile([P, HL, CC, DV], BF16, name="wvct_sb",
                             tag="wvct_sb")
        eps_sb = const.tile([P, 1], F32, name="eps_sb", tag="eps_sb")
        nc.vector.memset(eps_sb, 1e-5)

        def emit_late_weights():
            for hl in range(HL):
                nc.scalar.dma_start(out=wot_sb[hl],
                                    in_=wot[hl * P:(hl + 1) * P, :])
            nc.sync.dma_start(out=mask_sb, in_=maskt[:, :])
            nc.gpsimd.dma_start(
                out=wkcT_sb,
                in_=wkcT[:, :, :].rearrange("h (cc p) d -> p h cc d", p=P))
            nc.gpsimd.dma_start(
                out=wvct_sb,
                in_=wvct[:, :, :].rearrange("h (cc p) v -> p h cc v", p=P))

        # x-stream pools at session scope so the next batch's loads prefetch
        # during the previous batch's attention phase
        xpool = ctx.enter_context(tc.tile_pool(name="xp", bufs=2))
        x8pool = ctx.enter_context(tc.tile_pool(name="x8p", bufs=2))

        def emit_p4(b):
            """Output projection for batch b (wo column slice per core)."""
            with ExitStack() as p4:
                o_pool = p4.enter_context(tc.tile_pool(name=f"op{b}", bufs=4))
                psO = p4.enter_context(
                    tc.tile_pool(name=f"psO{b}", bufs=3, space="PSUM"))
                for qc in range(b * TCH, (b + 1) * TCH):
                    osb = o_pool.tile([P, D], F16, name="osb", tag="osb")
                    for dg in range(D // 512):
                        op = psO.tile([P, 512], F32, name="op", tag="psO")
                        for hl in range(HL):
                            _mm(nc, op, outcT[hl][:, qc * P:(qc + 1) * P],
                                wot_sb[hl][:, dg * 512:(dg + 1) * 512],
                                start=(hl == 0), stop=(hl == HL - 1))
                        if dg % 2 == 0:
                            nc.vector.tensor_copy(
                                osb[:, dg * 512:(dg + 1) * 512], op)
                        else:
                            nc.scalar.copy(osb[:, dg * 512:(dg + 1) * 512], op)
                    if qc == B * TCH - 1:
                        # split the final row across three queues to shorten
                        # the end-of-kernel DMA drain
                        r = slice(qc * P, (qc + 1) * P)
                        nc.sync.dma_start(out=out[r, 0:768],
                                          in_=osb[:, 0:768])
                        nc.gpsimd.dma_start(out=out[r, 768:1536],
                                            in_=osb[:, 768:1536])
                        nc.scalar.dma_start(out=out[r, 1536:D],
                                            in_=osb[:, 1536:D])
                    else:
                        eng = nc.sync if qc % 2 == 0 else nc.gpsimd
                        eng.dma_start(out=out[qc * P:(qc + 1) * P, :], in_=osb)

        pending_p4 = None
        for b in range(B):
            with ExitStack() as bctx:
                bper = bctx.enter_context(tc.tile_pool(name=f"bper{b}", bufs=1))
                nopeT = [bper.tile([P, S], BF16, name=f"nopeT{b}{h}",
                                   tag=f"nopeT{h}")
                         for h in range(HL)]
                peT = [bper.tile([DR, S], BF16, name=f"peT{b}{h}", tag=f"peT{h}")
                       for h in range(HL)]
                kpeT = bper.tile([DR, S], BF16, name=f"kpeT{b}", tag="kpeT")
                kvcT = bper.tile([P, CC, S], BF16, name=f"kvcT{b}", tag="kvcT")
                # one-bank psum arena for all P1 transposes; lives at batch
                # scope so the last chunk's transposes can be emitted from P3
                tps = bctx.enter_context(
                    tc.tile_pool(name=f"tps{b}", bufs=1, space="PSUM"))
                # batch scope: the last chunk's kvbf/qn are read from P3
                kvbpool = bctx.enter_context(tc.tile_pool(name=f"kvb{b}",
                                                          bufs=2))
                qnpool = bctx.enter_context(tc.tile_pool(name=f"qn{b}",
                                                         bufs=2))

                def emit_transposes(tch, kvbf, qn):
                    tok0 = tch * P
                    tpx = tps.tile([P, 6, P], BF16, name="tpx", tag="tpx")
                    for cc in range(CC):
                        nc.tensor.transpose(
                            tpx[:, cc], kvbf[:, cc * P:(cc + 1) * P], identb)
                    nc.vector.tensor_copy(kvcT[:, :, tok0:tok0 + P],
                                          tpx[:, 0:CC])
                    nc.tensor.transpose(tpx[0:DR, 4], kvbf[:, C:C + DR],
                                        identb)
                    nc.tensor.transpose(tpx[DR:P, 4], qn[:, 0:DR], identb)
                    nc.tensor.transpose(tpx[0:DR, 5], qn[:, DR:2 * DR],
                                        identb)
                    nc.scalar.copy(kpeT[:, tok0:tok0 + P], tpx[0:DR, 4])
                    nc.vector.tensor_copy(peT[0][:, tok0:tok0 + P],
                                          tpx[DR:P, 4])
                    nc.scalar.copy(peT[1][:, tok0:tok0 + P], tpx[0:DR, 5])

                # ---------------- P1: projections ----------------
                deferred = None
                with ExitStack() as p1:
                    kvfpool = p1.enter_context(tc.tile_pool(name=f"kvf{b}",
                                                            bufs=2))
                    sqpool = p1.enter_context(tc.tile_pool(name=f"sq{b}", bufs=2))
                    tmp = p1.enter_context(tc.tile_pool(name=f"tmp{b}", bufs=4))
                    kvps = p1.enter_context(
                        tc.tile_pool(name=f"kvps{b}", bufs=2, space="PSUM"))
                    qps = p1.enter_context(
                        tc.tile_pool(name=f"qps{b}", bufs=1, space="PSUM"))
                    ntps = p1.enter_context(
                        tc.tile_pool(name=f"ntps{b}", bufs=2, space="PSUM"))

                    for tg in range(TCH // XG):
                        xrblk = xpool.tile([P, DC, XG * P], FP8, name="xrblk",
                                           tag="xrblk")
                        x8blk = x8pool.tile([P, DC, XG * P], FP8, name="x8blk",
                                            tag="x8blk")
                        g0 = b * S + tg * XG * P
                        xrin = xr8t[:, g0:g0 + XG * P]
                        x8in = x8t[:, g0:g0 + XG * P]
                        if b == 0 and tg == 0:
                            # quartered first transfers so the first
                            # projection matmuls can start sooner
                            qd = DC // 4
                            for qi in range(4):
                                nc.gpsimd.dma_start(
                                    out=x8blk[:, qi * qd:(qi + 1) * qd],
                                    in_=x8in[qi * qd * P:(qi + 1) * qd * P, :]
                                        .rearrange("(a p) t -> p a t", p=P))
                            nc.sync.dma_start(
                                out=xrblk[:, 0:hd],
                                in_=xrin[0:hd * P, :]
                                    .rearrange("(a p) t -> p a t", p=P))
                            nc.sync.dma_start(
                                out=xrblk[:, hd:DC],
                                in_=xrin[hd * P:D, :]
                                    .rearrange("(a p) t -> p a t", p=P))
                            emit_wkva_a()
                            emit_late_weights()
                        else:
                            nc.sync.dma_start(
                                out=xrblk,
                                in_=xrin.rearrange("(a p) t -> p a t", p=P))
                            nc.gpsimd.dma_start(
                                out=x8blk,
                                in_=x8in.rearrange("(a p) t -> p a t", p=P))
                        # q nope part, emitted directly transposed ([d,tok])
                        # via DoubleRow with tokens as the moving free dim
                        tg0 = tg * XG * P
                        for h in range(HL):
                            ntp = ntps.tile([P, XG * P], F32, name="ntp",
                                            tag="ntp")
                            for dh in range(DC // 2):
                                _mm8(nc, ntp,
                                     wqn_sb[:, 2 * dh:2 * dh + 2, h, :],
                                     x8blk[:, 2 * dh:2 * dh + 2, :],
                                     start=(dh == 0), stop=(dh == DC // 2 - 1))
                            if h == 0:
                                nc.scalar.mul(
                                    nopeT[h][:, tg0:tg0 + XG * P], ntp,
                                    1.0 / QS)
                            else:
                                nc.vector.tensor_scalar_mul(
                                    nopeT[h][:, tg0:tg0 + XG * P], ntp,
                                    1.0 / QS)
                        for ti in range(XG):
                            tch = tg * XG + ti
                            xvr = xrblk[:, :, ti * P:(ti + 1) * P]
                            xv8 = x8blk[:, :, ti * P:(ti + 1) * P]
                            # pass order A,C,B: the B weights and x-residual
                            # stream arrive last at startup
                            seq = (("A", xv8), ("C", xv8), ("B", xvr))

                            # ---- latent kv projection (3-pass fp8 DR) ----
                            kvc_ps = kvps.tile([P, C], F32, name="kvc_ps",
                                               tag="kvc", bufs=3)
                            for pi, (nm, xa) in enumerate(seq):
                                wsb = wkvc_sb[nm]
                                for dh in range(DC // 2):
                                    _mm8(nc, kvc_ps,
                                         xa[:, 2 * dh:2 * dh + 2, :],
                                         wsb[:, 2 * dh:2 * dh + 2, :],
                                         start=(pi == 0 and dh == 0),
                                         stop=(pi == 2 and dh == DC // 2 - 1))
                            kpe_ps = kvps.tile([P, DR], F32, name="kpe_ps",
                                               tag="kpep", bufs=1)
                            for pi, (nm, xa) in enumerate(seq):
                                wsb = wkpe_sb[nm]
                                for dh in range(DC // 2):
                                    _mm8(nc, kpe_ps,
                                         xa[:, 2 * dh:2 * dh + 2, :],
                                         wsb[:, 2 * dh:2 * dh + 2, :],
                                         start=(pi == 0 and dh == 0),
                                         stop=(pi == 2 and dh == DC // 2 - 1))
                            # ---- q rope-part projection (fp8 DoubleRow) ----
                            qp = qps.tile([P, HL * DR], F32, name="qp", tag="qp")
                            for dh in range(DC // 2):
                                _mm8(nc, qp, xv8[:, 2 * dh:2 * dh + 2, :],
                                     wq_sb[:, 2 * dh:2 * dh + 2, :],
                                     start=(dh == 0), stop=(dh == DC // 2 - 1))
                            # transposes of the PREVIOUS chunk (its LN/rope
                            # has had a full chunk of time to finish)
                            if deferred is not None:
                                emit_transposes(*deferred)

                            # ---- evict latent + fused layernorm sums on the
                            # Act accumulator (saves two DVE reduces) ----
                            kvf = kvfpool.tile([P, C + DR], F32, name="kvf",
                                               tag="kvf")
                            msum = tmp.tile([P, 1], F32, name="msum", tag="msum")
                            nc.scalar.activation(kvf[:, 0:C], kvc_ps, AF.Copy,
                                                 scale=1.0 / QS,
                                                 accum_out=msum)
                            nc.vector.tensor_scalar_mul(kvf[:, C:C + DR],
                                                        kpe_ps, 1.0 / QS)

                            mneg = tmp.tile([P, 1], F32, name="mneg", tag="mneg")
                            nc.scalar.mul(mneg, msum, -1.0 / C)
                            nc.gpsimd.tensor_scalar_add(kvf[:, 0:C],
                                                        kvf[:, 0:C], mneg)
                            sq = sqpool.tile([P, C], F32, name="sq", tag="sq")
                            var = tmp.tile([P, 1], F32, name="var", tag="var")
                            nc.scalar.activation(sq, kvf[:, 0:C], AF.Square,
                                                 accum_out=var)
                            std = tmp.tile([P, 1], F32, name="std", tag="std")
                            nc.scalar.activation(std, var, AF.Sqrt,
                                                 bias=eps_sb, scale=1.0 / C)
                            rstd = tmp.tile([P, 1], F32, name="rstd",
                                            tag="rstd")
                            nc.vector.reciprocal(rstd, std)
                            nc.vector.tensor_scalar_mul(kvf[:, 0:C],
                                                        kvf[:, 0:C], rstd)

                            # ---- rope on shared key pe ([evens|odds]) ----
                            cosv = cs_sb[:, tch, 0:DR // 2]
                            sinv = cs_sb[:, tch, DR // 2:DR]
                            ke, ko = kvf[:, C:C + 32], kvf[:, C + 32:C + DR]
                            t1 = tmp.tile([P, DR // 2], F32, name="t1", tag="t1")
                            t2 = tmp.tile([P, DR // 2], F32, name="t2", tag="t2")
                            t3 = tmp.tile([P, DR // 2], F32, name="t3", tag="t3")
                            t4 = tmp.tile([P, DR // 2], F32, name="t4", tag="t4")
                            nc.vector.tensor_mul(t1, ke, cosv)
                            nc.vector.tensor_mul(t2, ko, sinv)
                            nc.vector.tensor_mul(t3, ke, sinv)
                            nc.vector.tensor_mul(t4, ko, cosv)
                            nc.vector.tensor_sub(ke, t1, t2)
                            nc.vector.tensor_add(ko, t3, t4)

                            # ---- q eviction (undo fp8 range scale) + rope ----
                            qn = qnpool.tile([P, HL * DR], BF16, name="qn",
                                             tag="qn")
                            nc.scalar.mul(qn, qp, 1.0 / QS)
                            for h in range(HL):
                                o = h * DR
                                qe, qo = qn[:, o:o + 32], qn[:, o + 32:o + DR]
                                eng = nc.vector if h == 0 else nc.gpsimd
                                u1 = tmp.tile([P, DR // 2], BF16, name="u1",
                                              tag=f"u1{h}")
                                u2 = tmp.tile([P, DR // 2], BF16, name="u2",
                                              tag=f"u2{h}")
                                u3 = tmp.tile([P, DR // 2], BF16, name="u3",
                                              tag=f"u3{h}")
                                u4 = tmp.tile([P, DR // 2], BF16, name="u4",
                                              tag=f"u4{h}")
                                eng.tensor_mul(u1, qe, cosv)
                                eng.tensor_mul(u2, qo, sinv)
                                eng.tensor_mul(u3, qe, sinv)
                                eng.tensor_mul(u4, qo, cosv)
                                eng.tensor_sub(qe, u1, u2)
                                eng.tensor_add(qo, u3, u4)

                            # ---- round latent+kpe to bf16 for transposes ----
                            kvbf = kvbpool.tile([P, C + DR], BF16, name="kvbf",
                                                tag="kvbf")
                            nc.vector.tensor_copy(kvbf, kvf)
                            deferred = (tch, kvbf, qn)

                # batch 0's output projection is deferred to here so its DMAs
                # overlap batch 1's compute
                if pending_p4 is not None:
                    emit_p4(pending_p4)

                # ---------------- P3: attention ----------------
                with ExitStack() as p3:
                    kt_p = p3.enter_context(tc.tile_pool(name=f"ktp{b}",
                                                         bufs=2))
                    v_p = p3.enter_context(tc.tile_pool(name=f"vp{b}", bufs=2))
                    ex_p = p3.enter_context(tc.tile_pool(name=f"ex{b}", bufs=8))
                    sm_p = p3.enter_context(tc.tile_pool(name=f"smp{b}", bufs=2))
                    rb_p = p3.enter_context(tc.tile_pool(name=f"rbp{b}", bufs=2))
                    # shared psum pools across both heads (avoids per-head
                    # pool churn); KT shares the scores tag, V the ctx tag
                    spps = p3.enter_context(
                        tc.tile_pool(name=f"sp{b}", bufs=3, space="PSUM"))
                    ctxps = p3.enter_context(
                        tc.tile_pool(name=f"ctx{b}", bufs=2, space="PSUM"))
                    sumps = p3.enter_context(
                        tc.tile_pool(name=f"sum{b}", bufs=2, space="PSUM"))
                    # rtp shares the (idle-in-P3) transpose pool's bank slot

                    # dummy exp: pull the Sqrt->Exp act-table reload off the
                    # critical path (overlaps the K^T/V matmuls below)
                    junk = sm_p.tile([P, 1], F32, name="junk", tag="junk")
                    nc.scalar.activation(junk, eps_sb, AF.Exp)

                    KT_h = {}
                    V_h = {}
                    for h in range(HL):
                        # ---- materialize per-head K^T and V ----
                        # (both heads first: ~14us of PE cover that lets the
                        # Act/DVE backlog from P1 drain before the first
                        # exp is on the critical path)
                        KT_sb = kt_p.tile([P, S], BF16, name="KT_sb",
                                          tag="KT_sb")
                        V_sb = v_p.tile([P, TCH, DV], BF16, name="V_sb",
                                        tag="V_sb")
                        KT_h[h] = KT_sb
                        V_h[h] = V_sb

                        def emit_kt(qt):
                            ktp = spps.tile([P, BLKQ], F32, name="ktp",
                                            tag="sp")
                            for cc in range(CC):
                                _mm(nc, ktp, wkcT_sb[:, h, cc],
                                    kvcT[:, cc, qt * BLKQ:(qt + 1) * BLKQ],
                                    start=(cc == 0), stop=(cc == CC - 1))
                            if qt % 2 == 0:
                                nc.vector.tensor_copy(
                                    KT_sb[:, qt * BLKQ:(qt + 1) * BLKQ], ktp)
                            else:
                                nc.scalar.copy(
                                    KT_sb[:, qt * BLKQ:(qt + 1) * BLKQ], ktp)

                        def emit_v(t4):
                            vp4 = ctxps.tile([P, 4, DV], F32, name="vp4",
                                             tag="ctxp")
                            for j in range(4):
                                tc_i = t4 * 4 + j
                                for cc in range(CC):
                                    _mm(nc, vp4[:, j],
                                        kvcT[:, cc, tc_i * P:(tc_i + 1) * P],
                                        wvct_sb[:, h, cc],
                                        start=(cc == 0), stop=(cc == CC - 1))
                            if t4 % 2 == 0:
                                nc.scalar.copy(V_sb[:, t4 * 4:t4 * 4 + 4], vp4)
                            else:
                                nc.vector.tensor_copy(
                                    V_sb[:, t4 * 4:t4 * 4 + 4], vp4)

                        # the last key range (tokens 1536:2048) depends on the
                        # final P1 chunk's transposes; emit those only after
                        # ~5us of covering matmuls so the P1 LN/rope tail has
                        # drained by then
                        for qt in range(3):
                            emit_kt(qt)
                        for t4 in range(3):
                            emit_v(t4)
                        if deferred is not None:
                            emit_transposes(*deferred)
                            deferred = None
                        emit_kt(3)
                        emit_v(3)

                    for h in range(HL):
                        KT_sb = KT_h[h]
                        V_sb = V_h[h]
                        # blk3 first: its 12 leading key-chunks need no mask
                        # (off-diagonal), so the exp pipeline starts without
                        # waiting on the DVE backlog from P1
                        for blk in reversed(range(NBLK)):
                            nkc = (blk + 1) * (BLKQ // P)
                            q0 = blk * BLKQ
                            ctxp = ctxps.tile([P, BLKQ], F32,
                                              name="ctxp", tag="ctxp")
                            sums = sumps.tile([P, NBLK], F32, name="sums",
                                              tag="sums")
                            nc.vector.memset(sums, 0.0)

                            def consume(ex, kc, o):
                                for qc in range(o // P, NBLK):
                                    _mm(nc, sums[:, qc:qc + 1],
                                        ex[:, qc * P:(qc + 1) * P],
                                        ones_col, start=False, stop=False)
                                _mm(nc, ctxp[:, o:], V_sb[:, kc], ex[:, o:],
                                    start=(kc == 0), stop=(kc == nkc - 1))

                            pending = None
                            for kc in range(nkc):
                                k0 = kc * P
                                # causal triangle trim: queries < k0 are
                                # fully masked for this key chunk
                                o = max(0, k0 - q0)
                                sp = spps.tile([P, BLKQ], F32, name="sp",
                                               tag="sp")
                                _mm(nc, sp[:, o:], KT_sb[:, k0:k0 + P],
                                    nopeT[h][:, q0 + o:q0 + BLKQ],
                                    start=True, stop=False)
                                _mm(nc, sp[:, o:], kpeT[:, k0:k0 + P],
                                    peT[h][:, q0 + o:q0 + BLKQ],
                                    start=False, stop=True)
                                if k0 >= q0:
                                    # triangular mask on the diagonal chunk
                                    nc.vector.tensor_add(sp[:, o:o + P],
                                                         sp[:, o:o + P],
                                                         mask_sb)
                                ex = ex_p.tile([P, BLKQ], BF16, name="ex",
                                               tag="ex")
                                nc.scalar.activation(ex[:, o:], sp[:, o:],
                                                     AF.Exp)
                                if pending is not None:
                                    consume(*pending)
                                pending = (ex, kc, o)
                            consume(*pending)

                            # softmax 1/Z: [tok,1] sums -> row -> bcast
                            # (f16 keeps the PE transposes at 1 cyc/row with
                            # ~5e-4 relative rounding on the scale factor)
                            rec_col = sm_p.tile([P, NBLK], F16,
                                                name="rec_col", tag="rec_col")
                            with nc.allow_low_precision(
                                    reason="softmax scale in f16"):
                                for qc in range(NBLK):
                                    nc.vector.reciprocal(
                                        rec_col[:, qc:qc + 1],
                                        sums[:, qc:qc + 1])
                            rtp = tps.tile([1, BLKQ], F16, name="rtp",
                                           tag="tpx")
                            for qc in range(NBLK):
                                nc.tensor.transpose(
                                    rtp[:, qc * P:(qc + 1) * P],
                                    rec_col[:, qc:qc + 1], identh)
                            rec_row = sm_p.tile([1, BLKQ], F16,
                                                name="rec_row", tag="rec_row")
                            nc.vector.tensor_copy(rec_row, rtp)
                            recbc = rb_p.tile([P, BLKQ], F16,
                                              name="recbc", tag="recbc")
                            nc.gpsimd.partition_broadcast(
                                recbc, rec_row[0:1, :])
                            nc.vector.tensor_mul(
                                outcT[h][:, b * S + q0:b * S + q0 + BLKQ],
                                ctxp, recbc)
            pending_p4 = b

        emit_p4(pending_p4)
    nc.finalize()
    return nc


_cache = {}


def get_nc():
    if "nc" not in _cache:
        _cache["nc"] = build_nc()
    return _cache["nc"]


def _pe_perm():
    """[evens; odds] permutation of the 64 rope dims."""
    return np.concatenate([np.arange(0, DR, 2), np.arange(1, DR, 2)])


def make_in_maps(x, wq, wkv_a, kv_g, kv_b, wkv_b, wo, start_pos):
    """Host-side sharding/layout prep. Returns (in_maps, out_bias)."""
    import ml_dtypes
    BF = ml_dtypes.bfloat16
    F8 = ml_dtypes.float8_e4m3

    x = np.asarray(x, dtype=np.float32)
    wq = np.asarray(wq, dtype=np.float32)
    wkv_a = np.asarray(wkv_a, dtype=np.float32)
    kv_g = np.asarray(kv_g, dtype=np.float32)
    kv_b = np.asarray(kv_b, dtype=np.float32)
    wkv_b = np.asarray(wkv_b, dtype=np.float32)
    wo = np.asarray(wo, dtype=np.float32)
    sp = int(start_pos)
    perm = _pe_perm()

    x2d = x.reshape(B * S, D)
    x8 = x2d.astype(F8)
    xr8 = (16.0 * (x2d - x8.astype(np.float32))).astype(F8)
    x8t = np.ascontiguousarray(x8.T)
    xr8t = np.ascontiguousarray(xr8.T)

    pos = (sp + np.arange(S)).astype(np.float32)
    inv = 1.0 / (10000.0 ** (np.arange(0, DR, 2, dtype=np.float32) / DR))
    ang = pos[:, None] * inv
    cs = np.concatenate([np.cos(ang), np.sin(ang)], axis=1).astype(np.float32)

    kk = np.arange(P, dtype=np.int64)
    maskt = np.where(kk[:, None] <= kk[None, :], 0.0, NEG)
    maskt = maskt.astype(np.float32).astype(BF)

    # kv projection with pe rows permuted to [evens; odds]; 3-pass
    # split-precision fp8 factors at a shared x512 product scale:
    #   x*w ~ (x8*wA + xr8*wB + x8*wC) / 512
    wkva_p = wkv_a.copy()
    wkva_p[C:] = wkv_a[C:][perm]
    wkvaA = (QS * wkva_p).astype(F8)
    wkva_r = wkva_p - wkvaA.astype(np.float32) / QS
    wkvaB = (32.0 * wkva_p).astype(F8)
    wkvaC = (QS * wkva_r).astype(F8)

    wkvb = wkv_b.reshape(H, DN + DV, C)
    # fold layernorm gamma into the absorbed projections; beta contributes a
    # softmax-invariant score shift plus a constant output bias added on host
    wkc_all = wkvb[:, :DN, :] * kv_g[None, None, :]
    wvc_all = wkvb[:, DN:, :] * kv_g[None, None, :]
    bias_hv = (wkvb[:, DN:, :] @ kv_b).reshape(H * DV)
    out_bias = (bias_hv @ wo.T).astype(np.float32)

    in_maps = []
    for c in range(N_CORES):
        hs = slice(HL * c, HL * (c + 1))
        wq_h = wq.reshape(H, NH, D)[hs].copy()
        # permute pe rows per head, fold SCALE and the fp8 range boost
        wq_h[:, DN:] = wq_h[:, DN:][:, perm]
        wq_h = wq_h * (SCALE * QS)
        wqpe_h = wq_h[:, DN:].reshape(HL * DR, D)
        wqn_h = wq_h[:, :DN].reshape(HL * DN, D)
        in_maps.append({
            "x8t": x8t,
            "xr8t": xr8t,
            "wqpe8": np.ascontiguousarray(wqpe_h.T.astype(F8)),
            "wqn8": np.ascontiguousarray(wqn_h.T.astype(F8)),
            "wkvaA": np.ascontiguousarray(wkvaA.T),
            "wkvaB": np.ascontiguousarray(wkvaB.T),
            "wkvaC": np.ascontiguousarray(wkvaC.T),
            "wkcT": np.ascontiguousarray(
                np.swapaxes(wkc_all[hs], 1, 2).astype(BF)),
            "wvct": np.ascontiguousarray(
                np.swapaxes(wvc_all[hs], 1, 2).astype(BF)),
            "wot": np.ascontiguousarray(
                wo[:, HL * DV * c:HL * DV * (c + 1)].T.astype(BF)),
            "cs": cs,
            "maskt": maskt,
        })
    return in_maps, out_bias


def kernel(x, wq, wkv_a, kv_g, kv_b, wkv_b, wo, start_pos):
    from concourse.bass_utils import run_bass_kernel_spmd

    in_maps, out_bias = make_in_maps(x, wq, wkv_a, kv_g, kv_b, wkv_b, wo,
                                     start_pos)
    res = run_bass_kernel_spmd(get_nc(), in_maps, list(range(N_CORES)))
    acc = np.zeros((B * S, D), np.float64)
    for r in res.results:
        acc += r["out"]
    acc += out_bias[None, :]
    return acc.astype(np.float32).reshape(B, S, D)


# revision 101
# speedup vs baseline: 1.0184x; 1.0184x over previous
"""MLA (multi-head latent attention) forward on 8 TRN2 NeuronCores.

Strategy: tensor-parallel over heads (16 heads -> 2 per core), SPMD with
host-side combine of the 8 partial outputs (wo is column-sharded so partials
sum to the full output).

Kernel structure (per core):
  P1  streaming pass over x^T: latent-kv projection in bf16 (+layernorm
      with both reduction sums fused onto the Act accumulator, +rope on
      the shared key pe); q projection in fp8e4 with DoubleRow matmuls
      (4x bf16 throughput per the 0.5 cyc/row + 256-deep contraction; the
      x512 fp8 range scale is undone at PSUM eviction).  The q nope part
      is emitted directly transposed (tokens as the moving free dim); the
      remaining attention operands are transposed on the PE in bf16,
      packed into a single 1-bank PSUM tile per token chunk.
  P3  per head: materialize per-head K^T = wkc^T kv_c^T and V = kv_c wvc^T
      from the latent (cuts the score/context contraction from 576 to 192/
      128 vs the absorbed form), then block-causal attention in "scores
      transposed" [k,q] orientation with causal triangle-trim of the
      diagonal key chunks (processed largest-block-first so the leading
      pairs need no mask).  Softmax denominators come from [128,1]-output
      matmuls (~1 PE row each), reciprocals are transposed back to a row
      and broadcast across partitions on the idle GPSIMD engine, and the
      normalize is fused into the context eviction.
  P4  output projection with the per-core wo column slice in f16 output
      (host combines in fp64); batch 0's P4 is deferred until after batch
      1's P1 so its output DMAs overlap compute.

Rope pairs are host-permuted to [evens; odds] so all rope arithmetic runs
on contiguous column slices (consistently applied to q_pe and k_pe, which
leaves scores invariant).
"""

import numpy as np
from contextlib import ExitStack

import concourse.bass as bass
import concourse.tile as tile
from concourse import bacc, mybir

# ---------------- problem dims (hardcoded per contest contract) -------------
B, S, D = 2, 2048, 2048
H = 16
C = 512            # kv_lora_rank
DN, DR, DV = 128, 64, 128
NH = DN + DR       # 192
SCALE = float((DN + DR) ** -0.5)
NEG = -1e9

N_CORES = 8
HL = H // N_CORES  # 2 local heads
P = 128
DC = D // P        # 16 contraction chunks over D
TCH = S // P       # 16 token chunks per batch
NBLK = 4           # query blocks per batch
BLKQ = S // NBLK   # 512
CC = C // P        # 4 chunks over latent dim
XG = 4             # token chunks per x-block DMA
QS = 512.0         # fp8 range scale for the q projection

F32 = mybir.dt.float32
F16 = mybir.dt.float16
BF16 = mybir.dt.bfloat16
FP8 = mybir.dt.float8e4
AF = mybir.ActivationFunctionType
DROW = mybir.MatmulPerfMode.DoubleRow


def _mm(nc, out, lhsT, rhs, start, stop):
    nc.tensor.matmul(out, lhsT, rhs, start=start, stop=stop,
                     skip_group_check=True)


def _mm8(nc, out, lhsT, rhs, start, stop):
    nc.tensor.matmul(out, lhsT, rhs, start=start, stop=stop,
                     perf_mode=DROW, skip_group_check=True)


def build_nc():
    from concourse.masks import make_identity

    nc = bacc.Bacc("TRN2", target_bir_lowering=False, debug=False,
                   num_devices=N_CORES)
    x8t = nc.dram_tensor("x8t", [D, B * S], FP8, kind="ExternalInput")
    xr8t = nc.dram_tensor("xr8t", [D, B * S], FP8, kind="ExternalInput")
    wqpe8 = nc.dram_tensor("wqpe8", [D, HL * DR], FP8, kind="ExternalInput")
    wqn8 = nc.dram_tensor("wqn8", [D, HL * DN], FP8, kind="ExternalInput")
    # 3-pass split-precision kv weights: x*w ~ x8*wA + xr8*wB + x8*wC, all
    # products at a shared x512 output scale (undone at PSUM eviction);
    # reconstruction error ~1e-3, better than a bf16 matmul
    wkvaA = nc.dram_tensor("wkvaA", [D, C + DR], FP8, kind="ExternalInput")
    wkvaB = nc.dram_tensor("wkvaB", [D, C + DR], FP8, kind="ExternalInput")
    wkvaC = nc.dram_tensor("wkvaC", [D, C + DR], FP8, kind="ExternalInput")
    wkcT = nc.dram_tensor("wkcT", [HL, C, DN], BF16, kind="ExternalInput")
    wvct = nc.dram_tensor("wvct", [HL, C, DV], BF16, kind="ExternalInput")
    wot = nc.dram_tensor("wot", [HL * DV, D], BF16, kind="ExternalInput")
    cs = nc.dram_tensor("cs", [S, DR], F32, kind="ExternalInput")
    maskt = nc.dram_tensor("maskt", [P, P], BF16, kind="ExternalInput")
    # f16 output halves the out-DMA volume; the host combines in fp64 and
    # the ~0.05% rounding on per-core partials is well inside the error budget
    out = nc.dram_tensor("out", [B * S, D], F16, kind="ExternalOutput")

    with tile.TileContext(nc) as tc, ExitStack() as ctx:
        const = ctx.enter_context(tc.tile_pool(name="const", bufs=1))
        octpool = ctx.enter_context(tc.tile_pool(name="octpool", bufs=1))

        # weight / table DMAs, ordered so the P1-critical ones land first:
        #   Pool queue: wkva (split in halves) -> x-stream -> K/V weights
        #   Act queue:  wq8 -> wot -> tables
        #   SP queue:   x-stream + out
        hd = DC // 2
        qd4 = DC // 4
        wq_sb = octpool.tile([P, DC, HL * DR], FP8, name="wq_sb", tag="wq_sb")
        wqn_sb = octpool.tile([P, DC, HL, DN], FP8, name="wqn_sb", tag="wqn_sb")
        for hl in range(HL):
            nc.sync.dma_start(
                out=wqn_sb[:, :, hl, :],
                in_=wqn8[:, hl * DN:(hl + 1) * DN]
                    .rearrange("(a p) n -> p a n", p=P))
        nc.scalar.dma_start(out=wq_sb,
                            in_=wqpe8[:, :].rearrange("(a p) n -> p a n", p=P))
        # dense per-part weight tiles: DoubleRow moving APs must be dense
        # (column-sliced rhs lowers incorrectly on hardware).  The three
        # passes load from three different queues so the first x-stream
        # blocks aren't starved behind weight DMAs.
        wkvc_sb = {}
        wkpe_sb = {}
        for nm, src, eng in (("A", wkvaA, nc.gpsimd), ("B", wkvaB, nc.sync),
                             ("C", wkvaC, nc.scalar)):
            wkvc_sb[nm] = octpool.tile([P, DC, C], FP8, name=f"wkvc{nm}_sb",
                                       tag=f"wkvc{nm}_sb")
            wkpe_sb[nm] = octpool.tile([P, DC, DR], FP8, name=f"wkpe{nm}_sb",
                                       tag=f"wkpe{nm}_sb")
            if nm == "A":
                continue  # A loads after the first x8 block (see P1 loop)
            eng.dma_start(
                out=wkvc_sb[nm],
                in_=src[:, 0:C].rearrange("(a p) n -> p a n", p=P))
            eng.dma_start(
                out=wkpe_sb[nm],
                in_=src[:, C:C + DR].rearrange("(a p) n -> p a n", p=P))

        def emit_wkva_a():
            for qi in range(4):
                nc.gpsimd.dma_start(
                    out=wkvc_sb["A"][:, qi * qd4:(qi + 1) * qd4],
                    in_=wkvaA[qi * qd4 * P:(qi + 1) * qd4 * P, 0:C]
                        .rearrange("(a p) n -> p a n", p=P))
            nc.gpsimd.dma_start(
                out=wkpe_sb["A"],
                in_=wkvaA[:, C:C + DR].rearrange("(a p) n -> p a n", p=P))
        wot_sb = [octpool.tile([P, D], BF16, name=f"wot_sb{hl}", tag=f"wot{hl}")
                  for hl in range(HL)]
        outcT = [octpool.tile([P, B * S], BF16, name=f"outcT{hl}",
                              tag=f"outcT{hl}")
                 for hl in range(HL)]

        identb = const.tile([P, P], BF16, name="identb", tag="identb")
        make_identity(nc, identb)
        identh = const.tile([P, P], F16, name="identh", tag="identh")
        make_identity(nc, identh)
        ones_col = const.tile([P, 1], BF16, name="ones_col", tag="ones_col")
        nc.vector.memset(ones_col, 1.0)
        cs_sb = const.tile([P, TCH, DR], F32, name="cs_sb", tag="cs_sb")
        nc.scalar.dma_start(out=cs_sb,
                            in_=cs[:, :].rearrange("(a p) r -> p a r", p=P))
        # tables not needed until P3/P4: the DMAs are issued after the first
        # x-stream loads (emit_late_weights) so they don't delay startup
        mask_sb = const.tile([P, P], BF16, name="mask_sb", tag="mask_sb")
        wkcT_sb = const.tile([P, HL, CC, DN], BF16, name="wkcT_sb",
                             tag="wkcT_sb")
        wvct_sb = const.tile([P, HL, CC, DV], BF16, name="wvct_sb",
                             tag="wvct_sb")
        eps_sb = const.tile([P, 1], F32, name="eps_sb", tag="eps_sb")
        nc.vector.memset(eps_sb, 1e-5)

        def emit_late_weights():
            for hl in range(HL):
                nc.scalar.dma_start(out=wot_sb[hl],
                                    in_=wot[hl * P:(hl + 1) * P, :])
            nc.sync.dma_start(out=mask_sb, in_=maskt[:, :])
            nc.gpsimd.dma_start(
                out=wkcT_sb,
                in_=wkcT[:, :, :].rearrange("h (cc p) d -> p h cc d", p=P))
            nc.gpsimd.dma_start(
                out=wvct_sb,
                in_=wvct[:, :, :].rearrange("h (cc p) v -> p h cc v", p=P))

        # x-stream pools at session scope so the next batch's loads prefetch
        # during the previous batch's attention phase
        xpool = ctx.enter_context(tc.tile_pool(name="xp", bufs=2))
        x8pool = ctx.enter_context(tc.tile_pool(name="x8p", bufs=2))

        def emit_p4(b):
            """Output projection for batch b (wo column slice per core)."""
            with ExitStack() as p4:
                o_pool = p4.enter_context(tc.tile_pool(name=f"op{b}", bufs=4))
                psO = p4.enter_context(
                    tc.tile_pool(name=f"psO{b}", bufs=3, space="PSUM"))
                for qc in range(b * TCH, (b + 1) * TCH):
                    osb = o_pool.tile([P, D], F16, name="osb", tag="osb")
                    for dg in range(D // 512):
                        op = psO.tile([P, 512], F32, name="op", tag="psO")
                        for hl in range(HL):
                            _mm(nc, op, outcT[hl][:, qc * P:(qc + 1) * P],
                                wot_sb[hl][:, dg * 512:(dg + 1) * 512],
                                start=(hl == 0), stop=(hl == HL - 1))
                        if dg % 2 == 0:
                            nc.vector.tensor_copy(
                                osb[:, dg * 512:(dg + 1) * 512], op)
                        else:
                            nc.scalar.copy(osb[:, dg * 512:(dg + 1) * 512], op)
                    if qc == B * TCH - 1:
                        # split the final row across three queues to shorten
                        # the end-of-kernel DMA drain
                        r = slice(qc * P, (qc + 1) * P)
                        nc.sync.dma_start(out=out[r, 0:768],
                                          in_=osb[:, 0:768])
                        nc.gpsimd.dma_start(out=out[r, 768:1536],
                                            in_=osb[:, 768:1536])
                        nc.scalar.dma_start(out=out[r, 1536:D],
                                            in_=osb[:, 1536:D])
                    else:
                        eng = nc.sync if qc % 2 == 0 else nc.gpsimd
                        eng.dma_start(out=out[qc * P:(qc + 1) * P, :], in_=osb)

        pending_p4 = None
        for b in range(B):
            with ExitStack() as bctx:
                bper = bctx.enter_context(tc.tile_pool(name=f"bper{b}", bufs=1))
                nopeT = [bper.tile([P, S], BF16, name=f"nopeT{b}{h}",
                                   tag=f"nopeT{h}")
                         for h in range(HL)]
                peT = [bper.tile([DR, S], BF16, name=f"peT{b}{h}", tag=f"peT{h}")
                       for h in range(HL)]
                kpeT = bper.tile([DR, S], BF16, name=f"kpeT{b}", tag="kpeT")
                kvcT = bper.tile([P, CC, S], BF16, name=f"kvcT{b}", tag="kvcT")
                # one-bank psum arena for all P1 transposes; lives at batch
                # scope so the last chunk's transposes can be emitted from P3
                tps = bctx.enter_context(
                    tc.tile_pool(name=f"tps{b}", bufs=1, space="PSUM"))
                # batch scope: the last chunk's kvbf/qn are read from P3
                kvbpool = bctx.enter_context(tc.tile_pool(name=f"kvb{b}",
                                                          bufs=2))
                qnpool = bctx.enter_context(tc.tile_pool(name=f"qn{b}",
                                                         bufs=2))

                def emit_transposes(tch, kvbf, qn):
                    tok0 = tch * P
                    tpx = tps.tile([P, 6, P], BF16, name="tpx", tag="tpx")
                    for cc in range(CC):
                        nc.tensor.transpose(
                            tpx[:, cc], kvbf[:, cc * P:(cc + 1) * P], identb)
                    nc.vector.tensor_copy(kvcT[:, :, tok0:tok0 + P],
                                          tpx[:, 0:CC])
                    nc.tensor.transpose(tpx[0:DR, 4], kvbf[:, C:C + DR],
                                        identb)
                    nc.tensor.transpose(tpx[DR:P, 4], qn[:, 0:DR], identb)
                    nc.tensor.transpose(tpx[0:DR, 5], qn[:, DR:2 * DR],
                                        identb)
                    nc.scalar.copy(kpeT[:, tok0:tok0 + P], tpx[0:DR, 4])
                    nc.vector.tensor_copy(peT[0][:, tok0:tok0 + P],
                                          tpx[DR:P, 4])
                    nc.scalar.copy(peT[1][:, tok0:tok0 + P], tpx[0:DR, 5])

                # ---------------- P1: projections ----------------
                deferred = None
                with ExitStack() as p1:
                    kvfpool = p1.enter_context(tc.tile_pool(name=f"kvf{b}",
                                                            bufs=2))
                    sqpool = p1.enter_context(tc.tile_pool(name=f"sq{b}", bufs=2))
                    tmp = p1.enter_context(tc.tile_pool(name=f"tmp{b}", bufs=4))
                    kvps = p1.enter_context(
                        tc.tile_pool(name=f"kvps{b}", bufs=2, space="PSUM"))
                    qps = p1.enter_context(
                        tc.tile_pool(name=f"qps{b}", bufs=1, space="PSUM"))
                    ntps = p1.enter_context(
                        tc.tile_pool(name=f"ntps{b}", bufs=2, space="PSUM"))

                    for tg in range(TCH // XG):
                        xrblk = xpool.tile([P, DC, XG * P], FP8, name="xrblk",
                                           tag="xrblk")
                        x8blk = x8pool.tile([P, DC, XG * P], FP8, name="x8blk",
                                            tag="x8blk")
                        g0 = b * S + tg * XG * P
                        xrin = xr8t[:, g0:g0 + XG * P]
                        x8in = x8t[:, g0:g0 + XG * P]
                        if b == 0 and tg == 0:
                            # quartered first transfers so the first
                            # projection matmuls can start sooner
                            qd = DC // 4
                            for qi in range(4):
                                nc.gpsimd.dma_start(
                                    out=x8blk[:, qi * qd:(qi + 1) * qd],
                                    in_=x8in[qi * qd * P:(qi + 1) * qd * P, :]
                                        .rearrange("(a p) t -> p a t", p=P))
                            nc.sync.dma_start(
                                out=xrblk[:, 0:hd],
                                in_=xrin[0:hd * P, :]
                                    .rearrange("(a p) t -> p a t", p=P))
                            nc.sync.dma_start(
                                out=xrblk[:, hd:DC],
                                in_=xrin[hd * P:D, :]
                                    .rearrange("(a p) t -> p a t", p=P))
                            emit_wkva_a()
                            emit_late_weights()
                        else:
                            nc.sync.dma_start(
                                out=xrblk,
                                in_=xrin.rearrange("(a p) t -> p a t", p=P))
                            nc.gpsimd.dma_start(
                                out=x8blk,
                                in_=x8in.rearrange("(a p) t -> p a t", p=P))
                        # q nope part, emitted directly transposed ([d,tok])
                        # via DoubleRow with tokens as the moving free dim
                        tg0 = tg * XG * P
                        for h in range(HL):
                            ntp = ntps.tile([P, XG * P], F32, name="ntp",
                                            tag="ntp")
                            for dh in range(DC // 2):
                                _mm8(nc, ntp,
                                     wqn_sb[:, 2 * dh:2 * dh + 2, h, :],
                                     x8blk[:, 2 * dh:2 * dh + 2, :],
                                     start=(dh == 0), stop=(dh == DC // 2 - 1))
                            if h == 0:
                                nc.scalar.mul(
                                    nopeT[h][:, tg0:tg0 + XG * P], ntp,
                                    1.0 / QS)
                            else:
                                nc.vector.tensor_scalar_mul(
                                    nopeT[h][:, tg0:tg0 + XG * P], ntp,
                                    1.0 / QS)
                        for ti in range(XG):
                            tch = tg * XG + ti
                            xvr = xrblk[:, :, ti * P:(ti + 1) * P]
                            xv8 = x8blk[:, :, ti * P:(ti + 1) * P]
                            # pass order A,C,B: the B weights and x-residual
                            # stream arrive last at startup
                            seq = (("A", xv8), ("C", xv8), ("B", xvr))

                            # ---- latent kv projection (3-pass fp8 DR) ----
                            kvc_ps = kvps.tile([P, C], F32, name="kvc_ps",
                                               tag="kvc", bufs=3)
                            for pi, (nm, xa) in enumerate(seq):
                                wsb = wkvc_sb[nm]
                                for dh in range(DC // 2):
                                    _mm8(nc, kvc_ps,
                                         xa[:, 2 * dh:2 * dh + 2, :],
                                         wsb[:, 2 * dh:2 * dh + 2, :],
                                         start=(pi == 0 and dh == 0),
                                         stop=(pi == 2 and dh == DC // 2 - 1))
                            kpe_ps = kvps.tile([P, DR], F32, name="kpe_ps",
                                               tag="kpep", bufs=1)
                            for pi, (nm, xa) in enumerate(seq):
                                wsb = wkpe_sb[nm]
                                for dh in range(DC // 2):
                                    _mm8(nc, kpe_ps,
                                         xa[:, 2 * dh:2 * dh + 2, :],
                                         wsb[:, 2 * dh:2 * dh + 2, :],
                                         start=(pi == 0 and dh == 0),
                                         stop=(pi == 2 and dh == DC // 2 - 1))
                            # ---- q rope-part projection (fp8 DoubleRow) ----
                            qp = qps.tile([P, HL * DR], F32, name="qp", tag="qp")
                            for dh in range(DC // 2):
                                _mm8(nc, qp, xv8[:, 2 * dh:2 * dh + 2, :],
                                     wq_sb[:, 2 * dh:2 * dh + 2, :],
                                     start=(dh == 0), stop=(dh == DC // 2 - 1))
                            # transposes of the PREVIOUS chunk (its LN/rope
                            # has had a full chunk of time to finish)
                            if deferred is not None:
                                emit_transposes(*deferred)

                            # ---- evict latent + fused layernorm sums on the
                            # Act accumulator (saves two DVE reduces) ----
                            kvf = kvfpool.tile([P, C + DR], F32, name="kvf",
                                               tag="kvf")
                            msum = tmp.tile([P, 1], F32, name="msum", tag="msum")
                            nc.scalar.activation(kvf[:, 0:C], kvc_ps, AF.Copy,
                                                 scale=1.0 / QS,
                                                 accum_out=msum)
                            nc.vector.tensor_scalar_mul(kvf[:, C:C + DR],
                                                        kpe_ps, 1.0 / QS)

                            mneg = tmp.tile([P, 1], F32, name="mneg", tag="mneg")
                            nc.scalar.mul(mneg, msum, -1.0 / C)
                            nc.gpsimd.tensor_scalar_add(kvf[:, 0:C],
                                                        kvf[:, 0:C], mneg)
                            sq = sqpool.tile([P, C], F32, name="sq", tag="sq")
                            var = tmp.tile([P, 1], F32, name="var", tag="var")
                            nc.scalar.activation(sq, kvf[:, 0:C], AF.Square,
                                                 accum_out=var)
                            std = tmp.tile([P, 1], F32, name="std", tag="std")
                            nc.scalar.activation(std, var, AF.Sqrt,
                                                 bias=eps_sb, scale=1.0 / C)
                            rstd = tmp.tile([P, 1], F32, name="rstd",
                                            tag="rstd")
                            nc.vector.reciprocal(rstd, std)
                            nc.vector.tensor_scalar_mul(kvf[:, 0:C],
                                                        kvf[:, 0:C], rstd)

                            # ---- rope on shared key pe ([evens|odds]) ----
                            cosv = cs_sb[:, tch, 0:DR // 2]
                            sinv = cs_sb[:, tch, DR // 2:DR]
                            ke, ko = kvf[:, C:C + 32], kvf[:, C + 32:C + DR]
                            t1 = tmp.tile([P, DR // 2], F32, name="t1", tag="t1")
                            t2 = tmp.tile([P, DR // 2], F32, name="t2", tag="t2")
                            t3 = tmp.tile([P, DR // 2], F32, name="t3", tag="t3")
                            t4 = tmp.tile([P, DR // 2], F32, name="t4", tag="t4")
                            nc.vector.tensor_mul(t1, ke, cosv)
                            nc.vector.tensor_mul(t2, ko, sinv)
                            nc.vector.tensor_mul(t3, ke, sinv)
                            nc.vector.tensor_mul(t4, ko, cosv)
                            nc.vector.tensor_sub(ke, t1, t2)
                            nc.vector.tensor_add(ko, t3, t4)

                            # ---- q eviction (undo fp8 range scale) + rope ----
                            qn = qnpool.tile([P, HL * DR], BF16, name="qn",
                                             tag="qn")
                            nc.scalar.mul(qn, qp, 1.0 / QS)
                            for h in range(HL):
                                o = h * DR
                                qe, qo = qn[:, o:o + 32], qn[:, o + 32:o + DR]
                                eng = nc.vector if h == 0 else nc.gpsimd
                                u1 = tmp.tile([P, DR // 2], BF16, name="u1",
                                              tag=f"u1{h}")
                                u2 = tmp.tile([P, DR // 2], BF16, name="u2",
                                              tag=f"u2{h}")
                                u3 = tmp.tile([P, DR // 2], BF16, name="u3",
                                              tag=f"u3{h}")
                                u4 = tmp.tile([P, DR // 2], BF16, name="u4",
                                              tag=f"u4{h}")
                                eng.tensor_mul(u1, qe, cosv)
                                eng.tensor_mul(u2, qo, sinv)
                                eng.tensor_mul(u3, qe, sinv)
                                eng.tensor_mul(u4, qo, cosv)
                                eng.tensor_sub(qe, u1, u2)
                                eng.tensor_add(qo, u3, u4)

                            # ---- round latent+kpe to bf16 for transposes ----
                            kvbf = kvbpool.tile([P, C + DR], BF16, name="kvbf",
                                                tag="kvbf")
                            nc.vector.tensor_copy(kvbf, kvf)
                            deferred = (tch, kvbf, qn)

                # batch 0's output projection is deferred to here so its DMAs
                # overlap batch 1's compute
                if pending_p4 is not None:
                    emit_p4(pending_p4)

                # ---------------- P3: attention ----------------
                with ExitStack() as p3:
                    kt_p = p3.enter_context(tc.tile_pool(name=f"ktp{b}",
                                                         bufs=2))
                    v_p = p3.enter_context(tc.tile_pool(name=f"vp{b}", bufs=2))
                    ex_p = p3.enter_context(tc.tile_pool(name=f"ex{b}", bufs=8))
                    sm_p = p3.enter_context(tc.tile_pool(name=f"smp{b}", bufs=2))
                    rb_p = p3.enter_context(tc.tile_pool(name=f"rbp{b}", bufs=2))
                    # shared psum pools across both heads (avoids per-head
                    # pool churn); KT shares the scores tag, V the ctx tag
                    spps = p3.enter_context(
                        tc.tile_pool(name=f"sp{b}", bufs=4, space="PSUM"))
                    ctxps = p3.enter_context(
                        tc.tile_pool(name=f"ctx{b}", bufs=2, space="PSUM"))
                    sumps = p3.enter_context(
                        tc.tile_pool(name=f"sum{b}", bufs=1, space="PSUM"))
                    # rtp shares the (idle-in-P3) transpose pool's bank slot

                    # dummy exp: pull the Sqrt->Exp act-table reload off the
                    # critical path (overlaps the K^T/V matmuls below)
                    junk = sm_p.tile([P, 1], F32, name="junk", tag="junk")
                    nc.scalar.activation(junk, eps_sb, AF.Exp)

                    KT_h = {}
                    V_h = {}
                    for h in range(HL):
                        # ---- materialize per-head K^T and V ----
                        # (both heads first: ~14us of PE cover that lets the
                        # Act/DVE backlog from P1 drain before the first
                        # exp is on the critical path)
                        KT_sb = kt_p.tile([P, S], BF16, name="KT_sb",
                                          tag="KT_sb")
                        V_sb = v_p.tile([P, TCH, DV], BF16, name="V_sb",
                                        tag="V_sb")
                        KT_h[h] = KT_sb
                        V_h[h] = V_sb

                        def emit_kt(qt):
                            ktp = spps.tile([P, BLKQ], F32, name="ktp",
                                            tag="sp")
                            for cc in range(CC):
                                _mm(nc, ktp, wkcT_sb[:, h, cc],
                                    kvcT[:, cc, qt * BLKQ:(qt + 1) * BLKQ],
                                    start=(cc == 0), stop=(cc == CC - 1))
                            if qt % 2 == 0:
                                nc.vector.tensor_copy(
                                    KT_sb[:, qt * BLKQ:(qt + 1) * BLKQ], ktp)
                            else:
                                nc.scalar.copy(
                                    KT_sb[:, qt * BLKQ:(qt + 1) * BLKQ], ktp)

                        def emit_v(t4):
                            vp4 = ctxps.tile([P, 4, DV], F32, name="vp4",
                                             tag="ctxp")
                            for j in range(4):
                                tc_i = t4 * 4 + j
                                for cc in range(CC):
                                    _mm(nc, vp4[:, j],
                                        kvcT[:, cc, tc_i * P:(tc_i + 1) * P],
                                        wvct_sb[:, h, cc],
                                        start=(cc == 0), stop=(cc == CC - 1))
                            if t4 % 2 == 0:
                                nc.scalar.copy(V_sb[:, t4 * 4:t4 * 4 + 4], vp4)
                            else:
                                nc.vector.tensor_copy(
                                    V_sb[:, t4 * 4:t4 * 4 + 4], vp4)

                        # the last key range (tokens 1536:2048) depends on the
                        # final P1 chunk's transposes; emit those only after
                        # ~5us of covering matmuls so the P1 LN/rope tail has
                        # drained by then
                        for qt in range(3):
                            emit_kt(qt)
                        for t4 in range(3):
                            emit_v(t4)
                        if deferred is not None:
                            emit_transposes(*deferred)
                            deferred = None
                        emit_kt(3)
                        emit_v(3)

                    for h in range(HL):
                        KT_sb = KT_h[h]
                        V_sb = V_h[h]
                        # blk3 first: its 12 leading key-chunks need no mask
                        # (off-diagonal), so the exp pipeline starts without
                        # waiting on the DVE backlog from P1
                        for blk in reversed(range(NBLK)):
                            nkc = (blk + 1) * (BLKQ // P)
                            q0 = blk * BLKQ
                            ctxp = ctxps.tile([P, BLKQ], F32,
                                              name="ctxp", tag="ctxp")
                            sums = sumps.tile([P, NBLK], F32, name="sums",
                                              tag="sums")
                            nc.vector.memset(sums, 0.0)

                            def consume(ex, kc, o):
                                for qc in range(o // P, NBLK):
                                    _mm(nc, sums[:, qc:qc + 1],
                                        ex[:, qc * P:(qc + 1) * P],
                                        ones_col, start=False, stop=False)
                                _mm(nc, ctxp[:, o:], V_sb[:, kc], ex[:, o:],
                                    start=(kc == 0), stop=(kc == nkc - 1))

                            pending = None
                            for kc in range(nkc):
                                k0 = kc * P
                                # causal triangle trim: queries < k0 are
                                # fully masked for this key chunk
                                o = max(0, k0 - q0)
                                sp = spps.tile([P, BLKQ], F32, name="sp",
                                               tag="sp")
                                _mm(nc, sp[:, o:], KT_sb[:, k0:k0 + P],
                                    nopeT[h][:, q0 + o:q0 + BLKQ],
                                    start=True, stop=False)
                                _mm(nc, sp[:, o:], kpeT[:, k0:k0 + P],
                                    peT[h][:, q0 + o:q0 + BLKQ],
                                    start=False, stop=True)
                                if k0 >= q0:
                                    # triangular mask on the diagonal chunk
                                    nc.vector.tensor_add(sp[:, o:o + P],
                                                         sp[:, o:o + P],
                                                         mask_sb)
                                ex = ex_p.tile([P, BLKQ], BF16, name="ex",
                                               tag="ex")
                                nc.scalar.activation(ex[:, o:], sp[:, o:],
                                                     AF.Exp)
                                if pending is not None:
                                    consume(*pending)
                                pending = (ex, kc, o)
                            consume(*pending)

                            # softmax 1/Z: [tok,1] sums -> row -> bcast
                            # (f16 keeps the PE transposes at 1 cyc/row with
                            # ~5e-4 relative rounding on the scale factor)
                            rec_col = sm_p.tile([P, NBLK], F16,
                                                name="rec_col", tag="rec_col")
                            with nc.allow_low_precision(
                                    reason="softmax scale in f16"):
                                for qc in range(NBLK):
                                    nc.vector.reciprocal(
                                        rec_col[:, qc:qc + 1],
                                        sums[:, qc:qc + 1])
                            rtp = tps.tile([1, BLKQ], F16, name="rtp",
                                           tag="tpx")
                            for qc in range(NBLK):
                                nc.tensor.transpose(
                                    rtp[:, qc * P:(qc + 1) * P],
                                    rec_col[:, qc:qc + 1], identh)
                            rec_row = sm_p.tile([1, BLKQ], F16,
                                                name="rec_row", tag="rec_row")
                            nc.vector.tensor_copy(rec_row, rtp)
                            recbc = rb_p.tile([P, BLKQ], F16,
                                              name="recbc", tag="recbc")
                            nc.gpsimd.partition_broadcast(
                                recbc, rec_row[0:1, :])
                            nc.vector.tensor_mul(
                                outcT[h][:, b * S + q0:b * S + q0 + BLKQ],
                                ctxp, recbc)
            pending_p4 = b

        emit_p4(pending_p4)
    nc.finalize()
    return nc


_cache = {}


def get_nc():
    if "nc" not in _cache:
        _cache["nc"] = build_nc()
    return _cache["nc"]


def _pe_perm():
    """[evens; odds] permutation of the 64 rope dims."""
    return np.concatenate([np.arange(0, DR, 2), np.arange(1, DR, 2)])


def make_in_maps(x, wq, wkv_a, kv_g, kv_b, wkv_b, wo, start_pos):
    """Host-side sharding/layout prep. Returns (in_maps, out_bias)."""
    import ml_dtypes
    BF = ml_dtypes.bfloat16
    F8 = ml_dtypes.float8_e4m3

    x = np.asarray(x, dtype=np.float32)
    wq = np.asarray(wq, dtype=np.float32)
    wkv_a = np.asarray(wkv_a, dtype=np.float32)
    kv_g = np.asarray(kv_g, dtype=np.float32)
    kv_b = np.asarray(kv_b, dtype=np.float32)
    wkv_b = np.asarray(wkv_b, dtype=np.float32)
    wo = np.asarray(wo, dtype=np.float32)
    sp = int(start_pos)
    perm = _pe_perm()

    x2d = x.reshape(B * S, D)
    x8 = x2d.astype(F8)
    xr8 = (16.0 * (x2d - x8.astype(np.float32))).astype(F8)
    x8t = np.ascontiguousarray(x8.T)
    xr8t = np.ascontiguousarray(xr8.T)

    pos = (sp + np.arange(S)).astype(np.float32)
    inv = 1.0 / (10000.0 ** (np.arange(0, DR, 2, dtype=np.float32) / DR))
    ang = pos[:, None] * inv
    cs = np.concatenate([np.cos(ang), np.sin(ang)], axis=1).astype(np.float32)

    kk = np.arange(P, dtype=np.int64)
    maskt = np.where(kk[:, None] <= kk[None, :], 0.0, NEG)
    maskt = maskt.astype(np.float32).astype(BF)

    # kv projection with pe rows permuted to [evens; odds]; 3-pass
    # split-precision fp8 factors at a shared x512 product scale:
    #   x*w ~ (x8*wA + xr8*wB + x8*wC) / 512
    wkva_p = wkv_a.copy()
    wkva_p[C:] = wkv_a[C:][perm]
    wkvaA = (QS * wkva_p).astype(F8)
    wkva_r = wkva_p - wkvaA.astype(np.float32) / QS
    wkvaB = (32.0 * wkva_p).astype(F8)
    wkvaC = (QS * wkva_r).astype(F8)

    wkvb = wkv_b.reshape(H, DN + DV, C)
    # fold layernorm gamma into the absorbed projections; beta contributes a
    # softmax-invariant score shift plus a constant output bias added on host
    wkc_all = wkvb[:, :DN, :] * kv_g[None, None, :]
    wvc_all = wkvb[:, DN:, :] * kv_g[None, None, :]
    bias_hv = (wkvb[:, DN:, :] @ kv_b).reshape(H * DV)
    out_bias = (bias_hv @ wo.T).astype(np.float32)

    in_maps = []
    for c in range(N_CORES):
        hs = slice(HL * c, HL * (c + 1))
        wq_h = wq.reshape(H, NH, D)[hs].copy()
        # permute pe rows per head, fold SCALE and the fp8 range boost
        wq_h[:, DN:] = wq_h[:, DN:][:, perm]
        wq_h = wq_h * (SCALE * QS)
        wqpe_h = wq_h[:, DN:].reshape(HL * DR, D)
        wqn_h = wq_h[:, :DN].reshape(HL * DN, D)
        in_maps.append({
            "x8t": x8t,
            "xr8t": xr8t,
            "wqpe8": np.ascontiguousarray(wqpe_h.T.astype(F8)),
            "wqn8": np.ascontiguousarray(wqn_h.T.astype(F8)),
            "wkvaA": np.ascontiguousarray(wkvaA.T),
            "wkvaB": np.ascontiguousarray(wkvaB.T),
            "wkvaC": np.ascontiguousarray(wkvaC.T),
            "wkcT": np.ascontiguousarray(
                np.swapaxes(wkc_all[hs], 1, 2).astype(BF)),
            "wvct": np.ascontiguousarray(
                np.swapaxes(wvc_all[hs], 1, 2).astype(BF)),
            "wot": np.ascontiguousarray(
                wo[:, HL * DV * c:HL * DV * (c + 1)].T.astype(BF)),
            "cs": cs,
            "maskt": maskt,
        })
    return in_maps, out_bias


def kernel(x, wq, wkv_a, kv_g, kv_b, wkv_b, wo, start_pos):
    from concourse.bass_utils import run_bass_kernel_spmd

    in_maps, out_bias = make_in_maps(x, wq, wkv_a, kv_g, kv_b, wkv_b, wo,
                                     start_pos)
    res = run_bass_kernel_spmd(get_nc(), in_maps, list(range(N_CORES)))
    acc = np.zeros((B * S, D), np.float64)
    for r in res.results:
        acc += r["out"]
    acc += out_bias[None, :]
    return acc.astype(np.float32).reshape(B, S, D)


# revision 104
# speedup vs baseline: 1.0276x; 1.0090x over previous
"""MLA (multi-head latent attention) forward on 8 TRN2 NeuronCores.

Strategy: tensor-parallel over heads (16 heads -> 2 per core), SPMD with
host-side combine of the 8 partial outputs (wo is column-sharded so partials
sum to the full output).

Kernel structure (per core):
  P1  streaming pass over x^T: latent-kv projection in bf16 (+layernorm
      with both reduction sums fused onto the Act accumulator, +rope on
      the shared key pe); q projection in fp8e4 with DoubleRow matmuls
      (4x bf16 throughput per the 0.5 cyc/row + 256-deep contraction; the
      x512 fp8 range scale is undone at PSUM eviction).  The q nope part
      is emitted directly transposed (tokens as the moving free dim); the
      remaining attention operands are transposed on the PE in bf16,
      packed into a single 1-bank PSUM tile per token chunk.
  P3  per head: materialize per-head K^T = wkc^T kv_c^T and V = kv_c wvc^T
      from the latent (cuts the score/context contraction from 576 to 192/
      128 vs the absorbed form), then block-causal attention in "scores
      transposed" [k,q] orientation with causal triangle-trim of the
      diagonal key chunks (processed largest-block-first so the leading
      pairs need no mask).  Softmax denominators come from [128,1]-output
      matmuls (~1 PE row each), reciprocals are transposed back to a row
      and broadcast across partitions on the idle GPSIMD engine, and the
      normalize is fused into the context eviction.
  P4  output projection with the per-core wo column slice in f16 output
      (host combines in fp64); batch 0's P4 is deferred until after batch
      1's P1 so its output DMAs overlap compute.

Rope pairs are host-permuted to [evens; odds] so all rope arithmetic runs
on contiguous column slices (consistently applied to q_pe and k_pe, which
leaves scores invariant).
"""

import numpy as np
from contextlib import ExitStack

import concourse.bass as bass
import concourse.tile as tile
from concourse import bacc, mybir

# ---------------- problem dims (hardcoded per contest contract) -------------
B, S, D = 2, 2048, 2048
H = 16
C = 512            # kv_lora_rank
DN, DR, DV = 128, 64, 128
NH = DN + DR       # 192
SCALE = float((DN + DR) ** -0.5)
NEG = -1e9

N_CORES = 8
HL = H // N_CORES  # 2 local heads
P = 128
DC = D // P        # 16 contraction chunks over D
TCH = S // P       # 16 token chunks per batch
NBLK = 4           # query blocks per batch
BLKQ = S // NBLK   # 512
CC = C // P        # 4 chunks over latent dim
XG = 4             # token chunks per x-block DMA
QS = 512.0         # fp8 range scale for the q projection

F32 = mybir.dt.float32
F16 = mybir.dt.float16
BF16 = mybir.dt.bfloat16
FP8 = mybir.dt.float8e4
AF = mybir.ActivationFunctionType
DROW = mybir.MatmulPerfMode.DoubleRow


def _mm(nc, out, lhsT, rhs, start, stop):
    nc.tensor.matmul(out, lhsT, rhs, start=start, stop=stop,
                     skip_group_check=True)


def _mm8(nc, out, lhsT, rhs, start, stop):
    nc.tensor.matmul(out, lhsT, rhs, start=start, stop=stop,
                     perf_mode=DROW, skip_group_check=True)


def build_nc():
    from concourse.masks import make_identity

    nc = bacc.Bacc("TRN2", target_bir_lowering=False, debug=False,
                   num_devices=N_CORES)
    x8t = nc.dram_tensor("x8t", [D, B * S], FP8, kind="ExternalInput")
    xr8t = nc.dram_tensor("xr8t", [D, B * S], FP8, kind="ExternalInput")
    wqpe8 = nc.dram_tensor("wqpe8", [D, HL * DR], FP8, kind="ExternalInput")
    wqn8 = nc.dram_tensor("wqn8", [D, HL * DN], FP8, kind="ExternalInput")
    # 3-pass split-precision kv weights: x*w ~ x8*wA + xr8*wB + x8*wC, all
    # products at a shared x512 output scale (undone at PSUM eviction);
    # reconstruction error ~1e-3, better than a bf16 matmul
    wkvaA = nc.dram_tensor("wkvaA", [D, C + DR], FP8, kind="ExternalInput")
    wkvaB = nc.dram_tensor("wkvaB", [D, C + DR], FP8, kind="ExternalInput")
    wkvaC = nc.dram_tensor("wkvaC", [D, C + DR], FP8, kind="ExternalInput")
    wkcT = nc.dram_tensor("wkcT", [HL, C, DN], BF16, kind="ExternalInput")
    wvct = nc.dram_tensor("wvct", [HL, C, DV], BF16, kind="ExternalInput")
    wot = nc.dram_tensor("wot", [HL * DV, D], BF16, kind="ExternalInput")
    cs = nc.dram_tensor("cs", [S, DR], F32, kind="ExternalInput")
    maskt = nc.dram_tensor("maskt", [P, P], BF16, kind="ExternalInput")
    # f16 output halves the out-DMA volume; the host combines in fp64 and
    # the ~0.05% rounding on per-core partials is well inside the error budget
    out = nc.dram_tensor("out", [B * S, D], F16, kind="ExternalOutput")

    with tile.TileContext(nc) as tc, ExitStack() as ctx:
        const = ctx.enter_context(tc.tile_pool(name="const", bufs=1))
        octpool = ctx.enter_context(tc.tile_pool(name="octpool", bufs=1))

        # weight / table DMAs, ordered so the P1-critical ones land first:
        #   Pool queue: wkva (split in halves) -> x-stream -> K/V weights
        #   Act queue:  wq8 -> wot -> tables
        #   SP queue:   x-stream + out
        hd = DC // 2
        qd4 = DC // 4
        wq_sb = octpool.tile([P, DC, HL * DR], FP8, name="wq_sb", tag="wq_sb")
        wqn_sb = octpool.tile([P, DC, HL, DN], FP8, name="wqn_sb", tag="wqn_sb")
        for hl in range(HL):
            nc.sync.dma_start(
                out=wqn_sb[:, :, hl, :],
                in_=wqn8[:, hl * DN:(hl + 1) * DN]
                    .rearrange("(a p) n -> p a n", p=P))
        nc.scalar.dma_start(out=wq_sb,
                            in_=wqpe8[:, :].rearrange("(a p) n -> p a n", p=P))
        # dense per-part weight tiles: DoubleRow moving APs must be dense
        # (column-sliced rhs lowers incorrectly on hardware).  The three
        # passes load from three different queues so the first x-stream
        # blocks aren't starved behind weight DMAs.
        wkvc_sb = {}
        wkpe_sb = {}
        for nm, src, eng in (("A", wkvaA, nc.gpsimd), ("B", wkvaB, nc.sync),
                             ("C", wkvaC, nc.scalar)):
            wkvc_sb[nm] = octpool.tile([P, DC, C], FP8, name=f"wkvc{nm}_sb",
                                       tag=f"wkvc{nm}_sb")
            wkpe_sb[nm] = octpool.tile([P, DC, DR], FP8, name=f"wkpe{nm}_sb",
                                       tag=f"wkpe{nm}_sb")
            if nm == "A":
                continue  # A loads after the first x8 block (see P1 loop)
            eng.dma_start(
                out=wkvc_sb[nm],
                in_=src[:, 0:C].rearrange("(a p) n -> p a n", p=P))
            eng.dma_start(
                out=wkpe_sb[nm],
                in_=src[:, C:C + DR].rearrange("(a p) n -> p a n", p=P))

        def emit_wkva_a():
            for qi in range(4):
                nc.gpsimd.dma_start(
                    out=wkvc_sb["A"][:, qi * qd4:(qi + 1) * qd4],
                    in_=wkvaA[qi * qd4 * P:(qi + 1) * qd4 * P, 0:C]
                        .rearrange("(a p) n -> p a n", p=P))
            nc.gpsimd.dma_start(
                out=wkpe_sb["A"],
                in_=wkvaA[:, C:C + DR].rearrange("(a p) n -> p a n", p=P))
        wot_sb = [octpool.tile([P, D], BF16, name=f"wot_sb{hl}", tag=f"wot{hl}")
                  for hl in range(HL)]
        outcT = [octpool.tile([P, B * S], BF16, name=f"outcT{hl}",
                              tag=f"outcT{hl}")
                 for hl in range(HL)]

        identb = const.tile([P, P], BF16, name="identb", tag="identb")
        make_identity(nc, identb)
        identh = const.tile([P, P], F16, name="identh", tag="identh")
        make_identity(nc, identh)
        ones_col = const.tile([P, 1], BF16, name="ones_col", tag="ones_col")
        nc.vector.memset(ones_col, 1.0)
        cs_sb = const.tile([P, TCH, DR], F32, name="cs_sb", tag="cs_sb")
        nc.scalar.dma_start(out=cs_sb,
                            in_=cs[:, :].rearrange("(a p) r -> p a r", p=P))
        # tables not needed until P3/P4: the DMAs are issued after the first
        # x-stream loads (emit_late_weights) so they don't delay startup
        mask_sb = const.tile([P, P], BF16, name="mask_sb", tag="mask_sb")
        wkcT_sb = const.tile([P, HL, CC, DN], BF16, name="wkcT_sb",
                             tag="wkcT_sb")
        wvct_sb = const.tile([P, HL, CC, DV], BF16, name="wvct_sb",
                             tag="wvct_sb")
        eps_sb = const.tile([P, 1], F32, name="eps_sb", tag="eps_sb")
        nc.vector.memset(eps_sb, 1e-5)

        def emit_late_weights():
            for hl in range(HL):
                nc.scalar.dma_start(out=wot_sb[hl],
                                    in_=wot[hl * P:(hl + 1) * P, :])
            nc.sync.dma_start(out=mask_sb, in_=maskt[:, :])
            nc.gpsimd.dma_start(
                out=wkcT_sb,
                in_=wkcT[:, :, :].rearrange("h (cc p) d -> p h cc d", p=P))
            nc.gpsimd.dma_start(
                out=wvct_sb,
                in_=wvct[:, :, :].rearrange("h (cc p) v -> p h cc v", p=P))

        # x-stream pools at session scope so the next batch's loads prefetch
        # during the previous batch's attention phase
        xpool = ctx.enter_context(tc.tile_pool(name="xp", bufs=2))
        x8pool = ctx.enter_context(tc.tile_pool(name="x8p", bufs=2))

        def emit_p4(b):
            """Output projection for batch b (wo column slice per core)."""
            with ExitStack() as p4:
                o_pool = p4.enter_context(tc.tile_pool(name=f"op{b}", bufs=4))
                psO = p4.enter_context(
                    tc.tile_pool(name=f"psO{b}", bufs=3, space="PSUM"))
                for qc in range(b * TCH, (b + 1) * TCH):
                    osb = o_pool.tile([P, D], F16, name="osb", tag="osb")
                    for dg in range(D // 512):
                        op = psO.tile([P, 512], F32, name="op", tag="psO")
                        for hl in range(HL):
                            _mm(nc, op, outcT[hl][:, qc * P:(qc + 1) * P],
                                wot_sb[hl][:, dg * 512:(dg + 1) * 512],
                                start=(hl == 0), stop=(hl == HL - 1))
                        if dg % 2 == 0:
                            nc.vector.tensor_copy(
                                osb[:, dg * 512:(dg + 1) * 512], op)
                        else:
                            nc.scalar.copy(osb[:, dg * 512:(dg + 1) * 512], op)
                    if qc == B * TCH - 1:
                        # split the final row across three queues to shorten
                        # the end-of-kernel DMA drain
                        r = slice(qc * P, (qc + 1) * P)
                        nc.sync.dma_start(out=out[r, 0:768],
                                          in_=osb[:, 0:768])
                        nc.gpsimd.dma_start(out=out[r, 768:1536],
                                            in_=osb[:, 768:1536])
                        nc.scalar.dma_start(out=out[r, 1536:D],
                                            in_=osb[:, 1536:D])
                    else:
                        eng = nc.sync if qc % 2 == 0 else nc.gpsimd
                        eng.dma_start(out=out[qc * P:(qc + 1) * P, :], in_=osb)

        pending_p4 = None
        for b in range(B):
            with ExitStack() as bctx:
                bper = bctx.enter_context(tc.tile_pool(name=f"bper{b}", bufs=1))
                nopeT = [bper.tile([P, S], BF16, name=f"nopeT{b}{h}",
                                   tag=f"nopeT{h}")
                         for h in range(HL)]
                peT = [bper.tile([DR, S], BF16, name=f"peT{b}{h}", tag=f"peT{h}")
                       for h in range(HL)]
                kpeT = bper.tile([DR, S], BF16, name=f"kpeT{b}", tag="kpeT")
                kvcT = bper.tile([P, CC, S], BF16, name=f"kvcT{b}", tag="kvcT")
                # batch scope: the last chunk's kvbf/qn are read from P3
                kvbpool = bctx.enter_context(tc.tile_pool(name=f"kvb{b}",
                                                          bufs=2))
                qnpool = bctx.enter_context(tc.tile_pool(name=f"qn{b}",
                                                         bufs=2))

                def emit_transposes(tch, kvbf, qn, pool, ptag):
                    tok0 = tch * P
                    tpx = pool.tile([P, 6, P], BF16, name="tpx",
                                    tag=ptag)
                    for cc in range(CC):
                        nc.tensor.transpose(
                            tpx[:, cc], kvbf[:, cc * P:(cc + 1) * P], identb)
                    nc.vector.tensor_copy(kvcT[:, :, tok0:tok0 + P],
                                          tpx[:, 0:CC])
                    nc.tensor.transpose(tpx[0:DR, 4], kvbf[:, C:C + DR],
                                        identb)
                    nc.tensor.transpose(tpx[DR:P, 4], qn[:, 0:DR], identb)
                    nc.tensor.transpose(tpx[0:DR, 5], qn[:, DR:2 * DR],
                                        identb)
                    nc.scalar.copy(kpeT[:, tok0:tok0 + P], tpx[0:DR, 4])
                    nc.vector.tensor_copy(peT[0][:, tok0:tok0 + P],
                                          tpx[DR:P, 4])
                    nc.scalar.copy(peT[1][:, tok0:tok0 + P], tpx[0:DR, 5])

                # ---------------- P1: projections ----------------
                deferred = None
                with ExitStack() as p1:
                    kvfpool = p1.enter_context(tc.tile_pool(name=f"kvf{b}",
                                                            bufs=2))
                    sqpool = p1.enter_context(tc.tile_pool(name=f"sq{b}", bufs=2))
                    tmp = p1.enter_context(tc.tile_pool(name=f"tmp{b}", bufs=4))
                    kvps = p1.enter_context(
                        tc.tile_pool(name=f"kvps{b}", bufs=2, space="PSUM"))
                    qps = p1.enter_context(
                        tc.tile_pool(name=f"qps{b}", bufs=1, space="PSUM"))
                    ntps = p1.enter_context(
                        tc.tile_pool(name=f"ntps{b}", bufs=2, space="PSUM"))
                    # one-bank arena for the P1 transposes (the final
                    # chunk's transposes, emitted from P3, borrow a scores
                    # slot instead)
                    tps = p1.enter_context(
                        tc.tile_pool(name=f"tps{b}", bufs=1, space="PSUM"))

                    for tg in range(TCH // XG):
                        xrblk = xpool.tile([P, DC, XG * P], FP8, name="xrblk",
                                           tag="xrblk")
                        x8blk = x8pool.tile([P, DC, XG * P], FP8, name="x8blk",
                                            tag="x8blk")
                        g0 = b * S + tg * XG * P
                        xrin = xr8t[:, g0:g0 + XG * P]
                        x8in = x8t[:, g0:g0 + XG * P]
                        if b == 0 and tg == 0:
                            # quartered first transfers so the first
                            # projection matmuls can start sooner
                            qd = DC // 4
                            for qi in range(4):
                                nc.gpsimd.dma_start(
                                    out=x8blk[:, qi * qd:(qi + 1) * qd],
                                    in_=x8in[qi * qd * P:(qi + 1) * qd * P, :]
                                        .rearrange("(a p) t -> p a t", p=P))
                            nc.sync.dma_start(
                                out=xrblk[:, 0:hd],
                                in_=xrin[0:hd * P, :]
                                    .rearrange("(a p) t -> p a t", p=P))
                            nc.sync.dma_start(
                                out=xrblk[:, hd:DC],
                                in_=xrin[hd * P:D, :]
                                    .rearrange("(a p) t -> p a t", p=P))
                            emit_wkva_a()
                            emit_late_weights()
                        else:
                            nc.sync.dma_start(
                                out=xrblk,
                                in_=xrin.rearrange("(a p) t -> p a t", p=P))
                            nc.gpsimd.dma_start(
                                out=x8blk,
                                in_=x8in.rearrange("(a p) t -> p a t", p=P))
                        # q nope part, emitted directly transposed ([d,tok])
                        # via DoubleRow with tokens as the moving free dim
                        tg0 = tg * XG * P
                        for h in range(HL):
                            ntp = ntps.tile([P, XG * P], F32, name="ntp",
                                            tag="ntp")
                            for dh in range(DC // 2):
                                _mm8(nc, ntp,
                                     wqn_sb[:, 2 * dh:2 * dh + 2, h, :],
                                     x8blk[:, 2 * dh:2 * dh + 2, :],
                                     start=(dh == 0), stop=(dh == DC // 2 - 1))
                            if h == 0:
                                nc.scalar.mul(
                                    nopeT[h][:, tg0:tg0 + XG * P], ntp,
                                    1.0 / QS)
                            else:
                                nc.vector.tensor_scalar_mul(
                                    nopeT[h][:, tg0:tg0 + XG * P], ntp,
                                    1.0 / QS)
                        for ti in range(XG):
                            tch = tg * XG + ti
                            xvr = xrblk[:, :, ti * P:(ti + 1) * P]
                            xv8 = x8blk[:, :, ti * P:(ti + 1) * P]
                            # pass order A,C,B: the B weights and x-residual
                            # stream arrive last at startup
                            seq = (("A", xv8), ("C", xv8), ("B", xvr))

                            # ---- latent kv projection (3-pass fp8 DR) ----
                            kvc_ps = kvps.tile([P, C], F32, name="kvc_ps",
                                               tag="kvc", bufs=3)
                            for pi, (nm, xa) in enumerate(seq):
                                wsb = wkvc_sb[nm]
                                for dh in range(DC // 2):
                                    _mm8(nc, kvc_ps,
                                         xa[:, 2 * dh:2 * dh + 2, :],
                                         wsb[:, 2 * dh:2 * dh + 2, :],
                                         start=(pi == 0 and dh == 0),
                                         stop=(pi == 2 and dh == DC // 2 - 1))
                            kpe_ps = kvps.tile([P, DR], F32, name="kpe_ps",
                                               tag="kpep", bufs=1)
                            for pi, (nm, xa) in enumerate(seq):
                                wsb = wkpe_sb[nm]
                                for dh in range(DC // 2):
                                    _mm8(nc, kpe_ps,
                                         xa[:, 2 * dh:2 * dh + 2, :],
                                         wsb[:, 2 * dh:2 * dh + 2, :],
                                         start=(pi == 0 and dh == 0),
                                         stop=(pi == 2 and dh == DC // 2 - 1))
                            # ---- q rope-part projection (fp8 DoubleRow) ----
                            qp = qps.tile([P, HL * DR], F32, name="qp", tag="qp")
                            for dh in range(DC // 2):
                                _mm8(nc, qp, xv8[:, 2 * dh:2 * dh + 2, :],
                                     wq_sb[:, 2 * dh:2 * dh + 2, :],
                                     start=(dh == 0), stop=(dh == DC // 2 - 1))
                            # transposes of the PREVIOUS chunk (its LN/rope
                            # has had a full chunk of time to finish)
                            if deferred is not None:
                                emit_transposes(*deferred, tps, "tpx")

                            # ---- evict latent + fused layernorm sums on the
                            # Act accumulator (saves two DVE reduces) ----
                            kvf = kvfpool.tile([P, C + DR], F32, name="kvf",
                                               tag="kvf")
                            msum = tmp.tile([P, 1], F32, name="msum", tag="msum")
                            nc.scalar.activation(kvf[:, 0:C], kvc_ps, AF.Copy,
                                                 scale=1.0 / QS,
                                                 accum_out=msum)
                            nc.vector.tensor_scalar_mul(kvf[:, C:C + DR],
                                                        kpe_ps, 1.0 / QS)

                            mneg = tmp.tile([P, 1], F32, name="mneg", tag="mneg")
                            nc.scalar.mul(mneg, msum, -1.0 / C)
                            nc.gpsimd.tensor_scalar_add(kvf[:, 0:C],
                                                        kvf[:, 0:C], mneg)
                            sq = sqpool.tile([P, C], F32, name="sq", tag="sq")
                            var = tmp.tile([P, 1], F32, name="var", tag="var")
                            nc.scalar.activation(sq, kvf[:, 0:C], AF.Square,
                                                 accum_out=var)
                            std = tmp.tile([P, 1], F32, name="std", tag="std")
                            nc.scalar.activation(std, var, AF.Sqrt,
                                                 bias=eps_sb, scale=1.0 / C)
                            rstd = tmp.tile([P, 1], F32, name="rstd",
                                            tag="rstd")
                            nc.vector.reciprocal(rstd, std)
                            nc.vector.tensor_scalar_mul(kvf[:, 0:C],
                                                        kvf[:, 0:C], rstd)

                            # ---- rope on shared key pe ([evens|odds]) ----
                            cosv = cs_sb[:, tch, 0:DR // 2]
                            sinv = cs_sb[:, tch, DR // 2:DR]
                            ke, ko = kvf[:, C:C + 32], kvf[:, C + 32:C + DR]
                            t1 = tmp.tile([P, DR // 2], F32, name="t1", tag="t1")
                            t2 = tmp.tile([P, DR // 2], F32, name="t2", tag="t2")
                            t3 = tmp.tile([P, DR // 2], F32, name="t3", tag="t3")
                            t4 = tmp.tile([P, DR // 2], F32, name="t4", tag="t4")
                            nc.vector.tensor_mul(t1, ke, cosv)
                            nc.vector.tensor_mul(t2, ko, sinv)
                            nc.vector.tensor_mul(t3, ke, sinv)
                            nc.vector.tensor_mul(t4, ko, cosv)
                            nc.vector.tensor_sub(ke, t1, t2)
                            nc.vector.tensor_add(ko, t3, t4)

                            # ---- q eviction (undo fp8 range scale) + rope ----
                            qn = qnpool.tile([P, HL * DR], BF16, name="qn",
                                             tag="qn")
                            nc.scalar.mul(qn, qp, 1.0 / QS)
                            for h in range(HL):
                                o = h * DR
                                qe, qo = qn[:, o:o + 32], qn[:, o + 32:o + DR]
                                eng = nc.vector if h == 0 else nc.gpsimd
                                u1 = tmp.tile([P, DR // 2], BF16, name="u1",
                                              tag=f"u1{h}")
                                u2 = tmp.tile([P, DR // 2], BF16, name="u2",
                                              tag=f"u2{h}")
                                u3 = tmp.tile([P, DR // 2], BF16, name="u3",
                                              tag=f"u3{h}")
                                u4 = tmp.tile([P, DR // 2], BF16, name="u4",
                                              tag=f"u4{h}")
                                eng.tensor_mul(u1, qe, cosv)
                                eng.tensor_mul(u2, qo, sinv)
                                eng.tensor_mul(u3, qe, sinv)
                                eng.tensor_mul(u4, qo, cosv)
                                eng.tensor_sub(qe, u1, u2)
                                eng.tensor_add(qo, u3, u4)

                            # ---- round latent+kpe to bf16 for transposes ----
                            kvbf = kvbpool.tile([P, C + DR], BF16, name="kvbf",
                                                tag="kvbf")
                            nc.vector.tensor_copy(kvbf, kvf)
                            deferred = (tch, kvbf, qn)

                # batch 0's output projection is deferred to here so its DMAs
                # overlap batch 1's compute
                if pending_p4 is not None:
                    emit_p4(pending_p4)

                # ---------------- P3: attention ----------------
                with ExitStack() as p3:
                    kt_p = p3.enter_context(tc.tile_pool(name=f"ktp{b}",
                                                         bufs=2))
                    v_p = p3.enter_context(tc.tile_pool(name=f"vp{b}", bufs=2))
                    ex_p = p3.enter_context(tc.tile_pool(name=f"ex{b}", bufs=8))
                    sm_p = p3.enter_context(tc.tile_pool(name=f"smp{b}", bufs=2))
                    rb_p = p3.enter_context(tc.tile_pool(name=f"rbp{b}", bufs=2))
                    # shared psum pools across both heads (avoids per-head
                    # pool churn); KT shares the scores tag, V the ctx tag
                    spps = p3.enter_context(
                        tc.tile_pool(name=f"sp{b}", bufs=5, space="PSUM"))
                    ctxps = p3.enter_context(
                        tc.tile_pool(name=f"ctx{b}", bufs=2, space="PSUM"))
                    sumps = p3.enter_context(
                        tc.tile_pool(name=f"sum{b}", bufs=1, space="PSUM"))
                    # rtp shares the (idle-in-P3) transpose pool's bank slot

                    # dummy exp: pull the Sqrt->Exp act-table reload off the
                    # critical path (overlaps the K^T/V matmuls below)
                    junk = sm_p.tile([P, 1], F32, name="junk", tag="junk")
                    nc.scalar.activation(junk, eps_sb, AF.Exp)

                    KT_h = {}
                    V_h = {}
                    for h in range(HL):
                        # ---- materialize per-head K^T and V ----
                        # (both heads first: ~14us of PE cover that lets the
                        # Act/DVE backlog from P1 drain before the first
                        # exp is on the critical path)
                        KT_sb = kt_p.tile([P, S], BF16, name="KT_sb",
                                          tag="KT_sb")
                        V_sb = v_p.tile([P, TCH, DV], BF16, name="V_sb",
                                        tag="V_sb")
                        KT_h[h] = KT_sb
                        V_h[h] = V_sb

                        def emit_kt(qt):
                            ktp = spps.tile([P, BLKQ], F32, name="ktp",
                                            tag="sp")
                            for cc in range(CC):
                                _mm(nc, ktp, wkcT_sb[:, h, cc],
                                    kvcT[:, cc, qt * BLKQ:(qt + 1) * BLKQ],
                                    start=(cc == 0), stop=(cc == CC - 1))
                            if qt % 2 == 0:
                                nc.vector.tensor_copy(
                                    KT_sb[:, qt * BLKQ:(qt + 1) * BLKQ], ktp)
                            else:
                                nc.scalar.copy(
                                    KT_sb[:, qt * BLKQ:(qt + 1) * BLKQ], ktp)

                        def emit_v(t4):
                            vp4 = ctxps.tile([P, 4, DV], F32, name="vp4",
                                             tag="ctxp")
                            for j in range(4):
                                tc_i = t4 * 4 + j
                                for cc in range(CC):
                                    _mm(nc, vp4[:, j],
                                        kvcT[:, cc, tc_i * P:(tc_i + 1) * P],
                                        wvct_sb[:, h, cc],
                                        start=(cc == 0), stop=(cc == CC - 1))
                            if t4 % 2 == 0:
                                nc.scalar.copy(V_sb[:, t4 * 4:t4 * 4 + 4], vp4)
                            else:
                                nc.vector.tensor_copy(
                                    V_sb[:, t4 * 4:t4 * 4 + 4], vp4)

                        # the last key range (tokens 1536:2048) depends on the
                        # final P1 chunk's transposes; emit those only after
                        # ~5us of covering matmuls so the P1 LN/rope tail has
                        # drained by then
                        for qt in range(3):
                            emit_kt(qt)
                        for t4 in range(3):
                            emit_v(t4)
                        if deferred is not None:
                            emit_transposes(*deferred, spps, "sp")
                            deferred = None
                        emit_kt(3)
                        emit_v(3)

                    for h in range(HL):
                        KT_sb = KT_h[h]
                        V_sb = V_h[h]
                        # blk3 first: its 12 leading key-chunks need no mask
                        # (off-diagonal), so the exp pipeline starts without
                        # waiting on the DVE backlog from P1
                        for blk in reversed(range(NBLK)):
                            nkc = (blk + 1) * (BLKQ // P)
                            q0 = blk * BLKQ
                            ctxp = ctxps.tile([P, BLKQ], F32,
                                              name="ctxp", tag="ctxp")
                            sums = sumps.tile([P, NBLK], F32, name="sums",
                                              tag="sums")
                            nc.vector.memset(sums, 0.0)

                            def consume(ex, kc, o):
                                for qc in range(o // P, NBLK):
                                    _mm(nc, sums[:, qc:qc + 1],
                                        ex[:, qc * P:(qc + 1) * P],
                                        ones_col, start=False, stop=False)
                                _mm(nc, ctxp[:, o:], V_sb[:, kc], ex[:, o:],
                                    start=(kc == 0), stop=(kc == nkc - 1))

                            pending = None
                            for kc in range(nkc):
                                k0 = kc * P
                                # causal triangle trim: queries < k0 are
                                # fully masked for this key chunk
                                o = max(0, k0 - q0)
                                sp = spps.tile([P, BLKQ], F32, name="sp",
                                               tag="sp")
                                _mm(nc, sp[:, o:], KT_sb[:, k0:k0 + P],
                                    nopeT[h][:, q0 + o:q0 + BLKQ],
                                    start=True, stop=False)
                                _mm(nc, sp[:, o:], kpeT[:, k0:k0 + P],
                                    peT[h][:, q0 + o:q0 + BLKQ],
                                    start=False, stop=True)
                                if k0 >= q0:
                                    # triangular mask on the diagonal chunk
                                    nc.vector.tensor_add(sp[:, o:o + P],
                                                         sp[:, o:o + P],
                                                         mask_sb)
                                ex = ex_p.tile([P, BLKQ], BF16, name="ex",
                                               tag="ex")
                                nc.scalar.activation(ex[:, o:], sp[:, o:],
                                                     AF.Exp)
                                if pending is not None:
                                    consume(*pending)
                                pending = (ex, kc, o)
                            consume(*pending)

                            # softmax 1/Z: [tok,1] sums -> row -> bcast
                            # (f16 keeps the PE transposes at 1 cyc/row with
                            # ~5e-4 relative rounding on the scale factor)
                            rec_col = sm_p.tile([P, NBLK], F16,
                                                name="rec_col", tag="rec_col")
                            with nc.allow_low_precision(
                                    reason="softmax scale in f16"):
                                for qc in range(NBLK):
                                    nc.vector.reciprocal(
                                        rec_col[:, qc:qc + 1],
                                        sums[:, qc:qc + 1])
                            rtp = sumps.tile([1, BLKQ], F16, name="rtp",
                                             tag="sums")
                            for qc in range(NBLK):
                                nc.tensor.transpose(
                                    rtp[:, qc * P:(qc + 1) * P],
                                    rec_col[:, qc:qc + 1], identh)
                            rec_row = sm_p.tile([1, BLKQ], F16,
                                                name="rec_row", tag="rec_row")
                            nc.vector.tensor_copy(rec_row, rtp)
                            recbc = rb_p.tile([P, BLKQ], F16,
                                              name="recbc", tag="recbc")
                            nc.gpsimd.partition_broadcast(
                                recbc, rec_row[0:1, :])
                            nc.vector.tensor_mul(
                                outcT[h][:, b * S + q0:b * S + q0 + BLKQ],
                                ctxp, recbc)
            pending_p4 = b

        emit_p4(pending_p4)
    nc.finalize()
    return nc


_cache = {}


def get_nc():
    if "nc" not in _cache:
        _cache["nc"] = build_nc()
    return _cache["nc"]


def _pe_perm():
    """[evens; odds] permutation of the 64 rope dims."""
    return np.concatenate([np.arange(0, DR, 2), np.arange(1, DR, 2)])


def make_in_maps(x, wq, wkv_a, kv_g, kv_b, wkv_b, wo, start_pos):
    """Host-side sharding/layout prep. Returns (in_maps, out_bias)."""
    import ml_dtypes
    BF = ml_dtypes.bfloat16
    F8 = ml_dtypes.float8_e4m3

    x = np.asarray(x, dtype=np.float32)
    wq = np.asarray(wq, dtype=np.float32)
    wkv_a = np.asarray(wkv_a, dtype=np.float32)
    kv_g = np.asarray(kv_g, dtype=np.float32)
    kv_b = np.asarray(kv_b, dtype=np.float32)
    wkv_b = np.asarray(wkv_b, dtype=np.float32)
    wo = np.asarray(wo, dtype=np.float32)
    sp = int(start_pos)
    perm = _pe_perm()

    x2d = x.reshape(B * S, D)
    x8 = x2d.astype(F8)
    xr8 = (16.0 * (x2d - x8.astype(np.float32))).astype(F8)
    x8t = np.ascontiguousarray(x8.T)
    xr8t = np.ascontiguousarray(xr8.T)

    pos = (sp + np.arange(S)).astype(np.float32)
    inv = 1.0 / (10000.0 ** (np.arange(0, DR, 2, dtype=np.float32) / DR))
    ang = pos[:, None] * inv
    cs = np.concatenate([np.cos(ang), np.sin(ang)], axis=1).astype(np.float32)

    kk = np.arange(P, dtype=np.int64)
    maskt = np.where(kk[:, None] <= kk[None, :], 0.0, NEG)
    maskt = maskt.astype(np.float32).astype(BF)

    # kv projection with pe rows permuted to [evens; odds]; 3-pass
    # split-precision fp8 factors at a shared x512 product scale:
    #   x*w ~ (x8*wA + xr8*wB + x8*wC) / 512
    wkva_p = wkv_a.copy()
    wkva_p[C:] = wkv_a[C:][perm]
    wkvaA = (QS * wkva_p).astype(F8)
    wkva_r = wkva_p - wkvaA.astype(np.float32) / QS
    wkvaB = (32.0 * wkva_p).astype(F8)
    wkvaC = (QS * wkva_r).astype(F8)

    wkvb = wkv_b.reshape(H, DN + DV, C)
    # fold layernorm gamma into the absorbed projections; beta contributes a
    # softmax-invariant score shift plus a constant output bias added on host
    wkc_all = wkvb[:, :DN, :] * kv_g[None, None, :]
    wvc_all = wkvb[:, DN:, :] * kv_g[None, None, :]
    bias_hv = (wkvb[:, DN:, :] @ kv_b).reshape(H * DV)
    out_bias = (bias_hv @ wo.T).astype(np.float32)

    in_maps = []
    for c in range(N_CORES):
        hs = slice(HL * c, HL * (c + 1))
        wq_h = wq.reshape(H, NH, D)[hs].copy()
        # permute pe rows per head, fold SCALE and the fp8 range boost
        wq_h[:, DN:] = wq_h[:, DN:][:, perm]
        wq_h = wq_h * (SCALE * QS)
        wqpe_h = wq_h[:, DN:].reshape(HL * DR, D)
        wqn_h = wq_h[:, :DN].reshape(HL * DN, D)
        in_maps.append({
            "x8t": x8t,
            "xr8t": xr8t,
            "wqpe8": np.ascontiguousarray(wqpe_h.T.astype(F8)),
            "wqn8": np.ascontiguousarray(wqn_h.T.astype(F8)),
            "wkvaA": np.ascontiguousarray(wkvaA.T),
            "wkvaB": np.ascontiguousarray(wkvaB.T),
            "wkvaC": np.ascontiguousarray(wkvaC.T),
            "wkcT": np.ascontiguousarray(
                np.swapaxes(wkc_all[hs], 1, 2).astype(BF)),
            "wvct": np.ascontiguousarray(
                np.swapaxes(wvc_all[hs], 1, 2).astype(BF)),
            "wot": np.ascontiguousarray(
                wo[:, HL * DV * c:HL * DV * (c + 1)].T.astype(BF)),
            "cs": cs,
            "maskt": maskt,
        })
    return in_maps, out_bias


def kernel(x, wq, wkv_a, kv_g, kv_b, wkv_b, wo, start_pos):
    from concourse.bass_utils import run_bass_kernel_spmd

    in_maps, out_bias = make_in_maps(x, wq, wkv_a, kv_g, kv_b, wkv_b, wo,
                                     start_pos)
    res = run_bass_kernel_spmd(get_nc(), in_maps, list(range(N_CORES)))
    acc = np.zeros((B * S, D), np.float64)
    for r in res.results:
        acc += r["out"]
    acc += out_bias[None, :]
    return acc.astype(np.float32).reshape(B, S, D)
